# revision 16
# baseline (speedup 1.0000x reference)
"""DecompFEDformerEncoder Trainium2 kernel.

Data-parallel over batch (B=32 -> 4 per core x 8 cores), full model per core.
Residual stream x kept in [D(part-chunks), T] layout, bf16 master + fp32 accums.
Moving-average decomposition via tensor_tensor_scan sliding window.
Fourier block: DFT-as-matmul (64 low modes), per-(h,m) complex einsum as
weight-stationary [128x128] matmuls (complex folded into K), PE transposes,
iDFT-as-matmul. Trend stream never materialized: mean-over-L harvested from
fused accum_out on eviction ops via the telescoping identity
  enc = mean(x_final) + sum_j (mean z_j - mean x_j) + (MA0-mean @ tw + biases).
"""
import sys, os

for _p in ('/opt/trn_rl_repo', '/root/.axon_site/_ro/trn_rl_repo'):
    if os.path.isdir(_p) and _p not in sys.path:
        sys.path.insert(0, _p)

import numpy as np
import ml_dtypes

import concourse.bass as bass
import concourse.mybir as mybir
import concourse.tile as tile
from concourse import bacc

BF16 = ml_dtypes.bfloat16
F32 = mybir.dt.float32
BF = mybir.dt.bfloat16
ALU = mybir.AluOpType
ACTF = mybir.ActivationFunctionType
AX = mybir.AxisListType

# problem dims
B, L, FEAT = 32, 1024, 128
D, DFF, H, NLAYERS = 512, 2048, 8, 2
MODES, K_MA, CLS = 64, 25, 128
E = D // H               # 64
NCORES = 8
BL = B // NCORES         # 4 batches per core
T = BL * L               # 4096 tokens per core
PAD = (K_MA - 1) // 2    # 12
SW = L + K_MA            # stripe width 1049
DCH = D // 128           # 4 chunks of channels
NPAIR = H * MODES        # 512 einsum pairs


# ---------------------------------------------------------------------------
# host-side constant staging
# ---------------------------------------------------------------------------

def _posemb():
    pos = np.arange(L, dtype=np.float64)[:, None]
    div = np.exp(np.arange(0, D, 2, dtype=np.float64) * (-np.log(10000.0) / D))
    pe = np.zeros((L, D), np.float64)
    pe[:, 0::2] = np.sin(pos * div)
    pe[:, 1::2] = np.cos(pos * div)
    return pe.astype(np.float32)


def build_consts(params, freq_index):
    """Return dict of replicated (same on all cores) dram input arrays."""
    g = lambda a: np.asarray(a)
    fi = np.asarray(freq_index).astype(np.int64)   # [64] mode indices
    c = {}

    # DFT matrix: F2[l, m] = cos(2 pi f_m l / L); F2[l, 64+m] = -sin(...)
    ll = np.arange(L, dtype=np.float64)[:, None]
    ang = 2.0 * np.pi * fi[None, :] * ll / L       # [L, 64]
    F2 = np.concatenate([np.cos(ang), -np.sin(ang)], axis=1)  # [1024, 128]
    # swizzle to [128, 8, 128]: F2h[p, k, m2] = F2[k*128+p, m2]
    F2h = F2.reshape(8, 128, 128).transpose(1, 0, 2).astype(BF16).copy()

    # iDFT: out[l] = sum_m s_m (Re[m] cos - Im[m] sin), s_m = (2 - [f_m==0])/L
    s = (2.0 - (fi == 0).astype(np.float64)) / L
    lr = np.arange(L, dtype=np.float64)[None, :]
    angi = 2.0 * np.pi * fi[:, None] * lr / L      # [64, L]
    G2 = np.stack([s[:, None] * np.cos(angi), -s[:, None] * np.sin(angi)], 1)
    # [64, 2, 1024] -> view [64, 2, 8, 128]
    c['G2'] = G2.reshape(64, 2, 8, 128).astype(BF16).copy()
    c['F2'] = F2h

    layers = params['layers']
    qw_h, ow_h, c1_h, c2_h, W2_h, qbc_h = [], [], [], [], [], []
    obsum = np.zeros(D, np.float64)
    for lp in layers:
        qw = g(lp['qw']).astype(np.float64)        # [512, 512]
        qb = g(lp['qb']).astype(np.float64)        # [512]
        ow = g(lp['ow']).astype(np.float64)
        ob = g(lp['ob']).astype(np.float64)
        fwre = g(lp['fw_re']).astype(np.float64)   # [8, 64, 64, 64] (h,i,o,m)
        fwim = g(lp['fw_im']).astype(np.float64)
        c1 = g(lp['c1']).astype(np.float64)        # [512, 2048]
        c2 = g(lp['c2']).astype(np.float64)        # [2048, 512]
        obsum += ob

        qw_h.append(qw.reshape(4, 128, 512).transpose(1, 0, 2).reshape(128, 4 * 512))
        ow_h.append(ow.reshape(4, 128, 512).transpose(1, 0, 2).reshape(128, 4 * 512))
        c1_h.append(c1.reshape(4, 128, DFF).transpose(1, 0, 2).reshape(128, 4 * DFF))
        c2_h.append(c2.reshape(16, 128, 512).transpose(1, 0, 2).reshape(128, 16 * 512))

        # einsum stationary: for pair p=(h*64+m): Wt[kappa=(r_in*64+i), (r_out*64+o)]
        # [[re,  im],
        #  [-im, re]]
        Wt = np.zeros((NPAIR, 128, 128), np.float64)
        fre = fwre.transpose(0, 3, 1, 2)           # [h, m, i, o]
        fim = fwim.transpose(0, 3, 1, 2)
        fre = fre.reshape(NPAIR, 64, 64)
        fim = fim.reshape(NPAIR, 64, 64)
        Wt[:, 0:64, 0:64] = fre
        Wt[:, 0:64, 64:128] = fim
        Wt[:, 64:128, 0:64] = -fim
        Wt[:, 64:128, 64:128] = fre
        # swizzle [32 blocks, 128(kappa), 16, 128]: W2h[bl, p, j, o2]=Wt[bl*16+j, p, o2]
        W2_h.append(Wt.reshape(32, 16, 128, 128).transpose(0, 2, 1, 3)
                      .reshape(32, 128, 16 * 128))

        # q-bias correction on X: modes with f_m == 0 get += L * qb (real part)
        qbc = np.zeros((128, H), np.float64)
        qbc[0:64, :] = (L * qb).reshape(H, E).T    # rows (re, i), col h
        qbc_h.append(qbc)

    c['qw'] = np.stack(qw_h).astype(BF16)
    c['ow'] = np.stack(ow_h).astype(BF16)
    c['c1'] = np.stack(c1_h).astype(BF16)
    c['c2'] = np.stack(c2_h).astype(BF16)
    c['W2'] = np.stack(W2_h).astype(BF16)          # [2, 32, 128, 2048]
    c['qbc'] = np.stack(qbc_h).astype(BF16)        # [2, 128, 8]

    sw = g(params['seas_w']).astype(np.float64)    # [128, 512]
    sb = g(params['seas_b']).astype(np.float64)    # [512]
    tw = g(params['trend_w']).astype(np.float64)
    tb = g(params['trend_b']).astype(np.float64)
    cw = g(params['cls_w']).astype(np.float64)     # [512, 128]
    cb = g(params['cls_b']).astype(np.float64)     # [128]

    c['sw'] = sw.astype(BF16)                      # [128, 512] lhsT natural
    c['sbb'] = sb.reshape(4, 128).T.astype(np.float32).copy()   # [128, 4]
    c['tw'] = tw.astype(np.float32)                # [128, 512]
    c['tbias'] = (tb + obsum).reshape(4, 128).T.astype(np.float32).copy()
    c['clsw'] = cw.reshape(4, 128, CLS).transpose(1, 0, 2).reshape(128, 4 * CLS).astype(np.float32).copy()
    c['clsb'] = np.broadcast_to(cb.astype(np.float32), (BL, CLS)).copy()
    c['pos'] = _posemb().T.reshape(4, 128, L).transpose(1, 0, 2).reshape(128, 4 * L).astype(BF16).copy()
    c['identb'] = np.eye(128, dtype=BF16)
    return c


def stage_core_inputs(x_enc):
    """Per-core input staging: x in [feat, T] bf16 + fp32 MA stripes."""
    maps = []
    for ci in range(NCORES):
        xs = np.asarray(x_enc[ci * BL:(ci + 1) * BL]).astype(np.float32)  # [4,1024,128]
        xT = xs.reshape(T, FEAT).T.copy()                  # [128, 4096]
        stripes = np.zeros((FEAT, BL, SW), np.float32)
        for b in range(BL):
            xb = xs[b].T                                    # [128, 1024]
            stripes[:, b, PAD + 1:PAD + 1 + L] = xb
            stripes[:, b, 0:PAD + 1] = xb[:, 0:1]
            stripes[:, b, PAD + 1 + L:] = xb[:, -1:]
        maps.append({'x_encT': xT.astype(BF16),
                     'stripes0': stripes.reshape(FEAT, BL * SW)})
    return maps


# ---------------------------------------------------------------------------
# device program
# ---------------------------------------------------------------------------

def build_program():
    nc = bacc.Bacc("TRN2", target_bir_lowering=False, debug=False)

    dram = {}
    def din(name, shape, dt):
        dram[name] = nc.dram_tensor(name, list(shape), dt, kind="ExternalInput")
        return dram[name]

    x_encT_d = din('x_encT', [FEAT, T], BF)
    stripes0_d = din('stripes0', [FEAT, BL * SW], F32)
    F2_d = din('F2', [128, 8, 128], BF)
    G2_d = din('G2', [64, 2, 8, 128], BF)
    qw_d = din('qw', [NLAYERS, 128, 4 * 512], BF)
    ow_d = din('ow', [NLAYERS, 128, 4 * 512], BF)
    c1_d = din('c1', [NLAYERS, 128, 4 * DFF], BF)
    c2_d = din('c2', [NLAYERS, 128, 16 * 512], BF)
    W2_d = din('W2', [NLAYERS, 32, 128, 16 * 128], BF)
    qbc_d = din('qbc', [NLAYERS, 128, H], BF)
    sw_d = din('sw', [128, 512], BF)
    sbb_d = din('sbb', [128, 4], F32)
    tw_d = din('tw', [128, 512], F32)
    tbias_d = din('tbias', [128, 4], F32)
    clsw_d = din('clsw', [128, 4 * CLS], F32)
    clsb_d = din('clsb', [BL, CLS], F32)
    pos_d = din('pos', [128, 4 * L], BF)
    identb_d = din('identb', [128, 128], BF)
    out_d = nc.dram_tensor('out', [BL, CLS], F32, kind="ExternalOutput")

    with tile.TileContext(nc) as tc:
        # ------- persistent pools
        with tc.tile_pool(name="resid", bufs=1) as resid_pool, \
             tc.tile_pool(name="consts", bufs=1) as cpool, \
             tc.tile_pool(name="slots", bufs=1) as spool, \
             tc.tile_pool(name="wpool", bufs=1) as wpool:

            xT = resid_pool.tile([128, DCH, T], BF)        # residual stream
            F2s = cpool.tile([128, 8, 128], BF)
            nc.sync.dma_start(F2s[:], F2_d[:])
            G2s = cpool.tile([64, 2, 8, 128], BF)
            nc.sync.dma_start(G2s[:], G2_d[:])
            sws = cpool.tile([128, 512], BF)
            nc.sync.dma_start(sws[:], sw_d[:])
            sbbs = cpool.tile([128, 4], F32)
            nc.sync.dma_start(sbbs[:], sbb_d[:])
            tws = cpool.tile([128, 512], F32)
            nc.sync.dma_start(tws[:], tw_d[:])
            tbs = cpool.tile([128, 4], F32)
            nc.sync.dma_start(tbs[:], tbias_d[:])
            clsws = cpool.tile([128, 4 * CLS], F32)
            nc.sync.dma_start(clsws[:], clsw_d[:])
            clsbs = cpool.tile([BL, CLS], F32)
            nc.sync.dma_start(clsbs[:], clsb_d[:])
            identb = cpool.tile([128, 128], BF)
            nc.sync.dma_start(identb[:], identb_d[:])

            # accumulation slots (all fp32, written exactly once each)
            # z-slots per (c,b): 0..3 attn(li*2+j), 4..7 ffn(li*2+half), 8 x0
            sl_z = spool.tile([128, DCH, BL, 9], F32)
            sl_x = spool.tile([128, DCH, BL, 4], F32)
            sl_seas0 = spool.tile([128, BL], F32)      # feat-space
            sl_xenc = spool.tile([128, BL], F32)       # feat-space

            # ---------------- stage 0: initial decomposition + embedding
            with tc.tile_pool(name="emb", bufs=1) as epool, \
                 tc.tile_pool(name="emb2", bufs=2) as epool2, \
                 tc.tile_pool(name="embps", bufs=2, space="PSUM") as eps:
                xe = epool.tile([128, T], BF)
                nc.sync.dma_start(xe[:], x_encT_d[:])
                str0 = epool.tile([128, BL, SW], F32)
                nc.sync.dma_start(str0[:], stripes0_d[:].rearrange(
                    "p (b s) -> p b s", b=BL))
                seas0 = epool.tile([128, T], BF)

                # sum_l x_enc per (feat, b)
                nc.vector.reduce_sum(sl_xenc[:], xe[:].rearrange(
                    "p (b l) -> p b l", b=BL), axis=AX.X)

                for b in range(BL):
                    init = epool2.tile([128, 1], F32, tag="init")
                    nc.vector.reduce_sum(init[:], str0[:, b, 0:K_MA], axis=AX.X)
                    scano = epool2.tile([128, L], F32, tag="scano")
                    nc.vector.tensor_tensor_scan(
                        scano[:], str0[:, b, K_MA:SW], str0[:, b, 0:L], init[:],
                        op0=ALU.add, op1=ALU.subtract)
                    # seas0 = x_enc - scano/25 ; accum = sum(seas0)
                    nc.vector.scalar_tensor_tensor(
                        out=seas0[:, b * L:(b + 1) * L], in0=scano[:],
                        scalar=-1.0 / K_MA, in1=xe[:, b * L:(b + 1) * L],
                        op0=ALU.mult, op1=ALU.add, accum_out=sl_seas0[:, b:b + 1])

                # embedding: xT[n-chunk] = sw.T @ seas0 + sb ; then += pos
                for n in range(DCH):
                    for s2 in range(8):
                        ps_t = eps.tile([128, 512], F32, tag="embps")
                        nc.tensor.matmul(ps_t[:], sws[:, n * 128:(n + 1) * 128],
                                         seas0[:, s2 * 512:(s2 + 1) * 512],
                                         start=True, stop=True)
                        nc.scalar.activation(xT[:, n, s2 * 512:(s2 + 1) * 512],
                                             ps_t[:], ACTF.Identity,
                                             bias=sbbs[:, n:n + 1])
                for n in range(DCH):
                    posc = epool2.tile([128, L], BF, tag="pos")
                    nc.sync.dma_start(posc[:], pos_d[:, n * L:(n + 1) * L])
                    for b in range(BL):
                        nc.vector.scalar_tensor_tensor(
                            out=xT[:, n, b * L:(b + 1) * L], in0=posc[:],
                            scalar=1.0, in1=xT[:, n, b * L:(b + 1) * L],
                            op0=ALU.mult, op1=ALU.add,
                            accum_out=sl_z[:, n, b, 8:9])   # slot: sum(x0)

            # ---------------- layers
            for li in range(NLAYERS):
                qws = wpool.tile([128, 4, 512], BF, tag="qw")
                nc.sync.dma_start(qws[:], qw_d[li].rearrange("p (k n) -> p k n", k=4))
                ows = wpool.tile([128, 4, 512], BF, tag="ow")
                nc.sync.dma_start(ows[:], ow_d[li].rearrange("p (k n) -> p k n", k=4))
                c1s = wpool.tile([128, 4, DFF], BF, tag="c1")
                nc.sync.dma_start(c1s[:], c1_d[li].rearrange("p (k n) -> p k n", k=4))
                c2s = wpool.tile([128, 16, 512], BF, tag="c2")
                nc.sync.dma_start(c2s[:], c2_d[li].rearrange("p (k n) -> p k n", k=16))
                qbcs = wpool.tile([128, H], BF, tag="qbc")
                nc.sync.dma_start(qbcs[:], qbc_d[li])

                _layer(nc, tc, li, xT, F2s, G2s, identb, qws, ows, c1s, c2s,
                       qbcs, W2_d, sl_z, sl_x)

            # ---------------- final: combine means, classifier
            with tc.tile_pool(name="fin", bufs=1) as fpool, \
                 tc.tile_pool(name="finps", bufs=2, space="PSUM") as fps:
                zsum = fpool.tile([128, DCH, BL], F32)
                nc.vector.reduce_sum(zsum[:], sl_z[:, :, :, 0:8], axis=AX.X)
                xsum = fpool.tile([128, DCH, BL], F32)
                nc.vector.reduce_sum(xsum[:], sl_x[:, :, :, 0:3], axis=AX.X)
                acc = fpool.tile([128, DCH, BL], F32)
                nc.vector.tensor_tensor(acc[:], zsum[:], xsum[:], op=ALU.subtract)

                # meanMA0 = (sum x_enc - sum seas0)/L  [feat, b]
                mm0 = fpool.tile([128, BL], F32)
                nc.vector.tensor_tensor(mm0[:], sl_xenc[:], sl_seas0[:], op=ALU.subtract)
                nc.vector.tensor_scalar_mul(mm0[:], mm0[:], 1.0 / L)

                enc = fpool.tile([128, DCH, BL], F32)
                for cch in range(DCH):
                    ps_tw = fps.tile([128, BL], F32, tag="twps")
                    nc.tensor.matmul(ps_tw[:], tws[:, cch * 128:(cch + 1) * 128],
                                     mm0[:], start=True, stop=True)
                    # enc = acc/L + tw-term
                    nc.vector.scalar_tensor_tensor(
                        out=enc[:, cch, :], in0=acc[:, cch, :], scalar=1.0 / L,
                        in1=ps_tw[:], op0=ALU.mult, op1=ALU.add)
                    nc.vector.tensor_scalar(
                        enc[:, cch, :], enc[:, cch, :], tbs[:, cch:cch + 1], None,
                        op0=ALU.add)

                ps_cls = fps.tile([BL, CLS], F32, tag="clsps")
                for cch in range(DCH):
                    nc.tensor.matmul(ps_cls[:], enc[:, cch, :],
                                     clsws[:, cch * CLS:(cch + 1) * CLS],
                                     start=(cch == 0), stop=(cch == DCH - 1))
                outs = fpool.tile([BL, CLS], F32)
                nc.vector.tensor_tensor(outs[:], ps_cls[:], clsbs[:], op=ALU.add)
                nc.sync.dma_start(out_d[:], outs[:])

    nc.compile()
    return nc


def _layer(nc, tc, li, xT, F2s, G2s, identb, qws, ows, c1s, c2s, qbcs,
           W2_d, sl_z, sl_x):
    """Emit one encoder layer."""
    # ============ fourier attention ============
    with tc.tile_pool(name=f"att{li}", bufs=1) as apool, \
         tc.tile_pool(name=f"att2{li}", bufs=2) as apool2:

        Xstk = apool.tile([128, NPAIR * BL], BF)          # [(r,i), (h,m,b)]
        X4 = Xstk[:].rearrange("p (h m b) -> p h m b", h=H, m=MODES, b=BL)

        # ---- q projection + DFT, per batch
        aps_cm = tc.tile_pool(name=f"attps{li}", bufs=2, space="PSUM")
        aps = aps_cm.__enter__()
        for b in range(BL):
            qsb = apool2.tile([128, 8, 512], BF, tag="qsb")
            for lc in range(8):
                tg = (b * 8 + lc) * 128
                ps_q = aps.tile([128, 512], F32, tag="qps")
                for k in range(DCH):
                    nc.tensor.matmul(ps_q[:], xT[:, k, tg:tg + 128], qws[:, k, :],
                                     start=(k == 0), stop=(k == DCH - 1))
                nc.scalar.activation(qsb[:, lc, :], ps_q[:], ACTF.Copy, bias=0.0)
            for cch in range(DCH):
                ps_x = aps.tile([128, 128], F32, tag="xps")
                for lc in range(8):
                    nc.tensor.matmul(ps_x[:], qsb[:, lc, cch * 128:(cch + 1) * 128],
                                     F2s[:, lc, :], start=(lc == 0), stop=(lc == 7))
                h0 = 2 * cch
                # re rows (0:64) <- psum cols 0:64 ; im rows (64:128) <- cols 64:128
                nc.vector.tensor_copy(X4[0:64, h0, :, b], ps_x[0:64, 0:64])
                nc.vector.tensor_copy(X4[0:64, h0 + 1, :, b], ps_x[64:128, 0:64])
                nc.vector.tensor_copy(X4[64:128, h0, :, b], ps_x[0:64, 64:128])
                nc.vector.tensor_copy(X4[64:128, h0 + 1, :, b], ps_x[64:128, 64:128])

        # q-bias correction on zero-frequency modes (mode list baked in host F2;
        # correction column m=0 matches freq_index arange convention)
        nc.vector.tensor_tensor(X4[:, :, 0, :], X4[:, :, 0, :],
                                qbcs[:, :, None].to_broadcast((128, H, BL)),
                                op=ALU.add)
        aps_cm.__exit__(None, None, None)

        # ---- einsum: 512 weight-stationary matmuls
        aps_cm = tc.tile_pool(name=f"attps{li}e", bufs=2, space="PSUM")
        aps = aps_cm.__enter__()
        OutSel = apool.tile([128, NPAIR * BL], BF)        # [(r,o), (pair,b)]
        for g4 in range(4):                                # psum bank groups
            ps_e = aps.tile([128, 512], F32, tag="eps")
            for bl in range(g4 * 8, (g4 + 1) * 8):         # 8 weight blocks of 16
                wsb = apool2.tile([128, 16 * 128], BF, tag="wsb")
                nc.sync.dma_start(wsb[:], W2_d[li, bl])
                for j in range(16):
                    p = bl * 16 + j
                    nc.tensor.matmul(ps_e[:, (p % 128) * 4:(p % 128) * 4 + 4],
                                     wsb[:, j * 128:(j + 1) * 128],
                                     Xstk[:, p * 4:(p + 1) * 4],
                                     start=True, stop=True)
            nc.vector.tensor_copy(OutSel[:, g4 * 512:(g4 + 1) * 512], ps_e[:])
        aps_cm.__exit__(None, None, None)

        # ---- transpose OutSel[(r,o),(h,m,b)] -> OutT[m,(r,h,o,b)]
        aps_cm = tc.tile_pool(name=f"attps{li}t", bufs=2, space="PSUM")
        aps = aps_cm.__enter__()
        OutT = apool.tile([64, 2 * H * E * BL], BF)
        OT4 = OutT[:].rearrange("p (r h o b) -> p r h o b", r=2, h=H, o=E, b=BL)
        OS4 = OutSel[:].rearrange("p (pr b) -> p pr b", b=BL)
        for hh in range(H):
            for b in range(BL):
                ps_t = aps.tile([64, 128], BF, tag="tps")
                nc.tensor.transpose(ps_t[:], OS4[:, hh * 64:(hh + 1) * 64, b],
                                    identb[:])
                nc.vector.tensor_copy(OT4[:, 0, hh, :, b], ps_t[:, 0:64])
                nc.vector.tensor_copy(OT4[:, 1, hh, :, b], ps_t[:, 64:128])
        aps_cm.__exit__(None, None, None)

        # ---- iDFT + o-projection + residual, per j-half
        aps_cm = tc.tile_pool(name=f"attps{li}v", bufs=2, space="PSUM")
        aps = aps_cm.__enter__()
        for j in range(2):
            Vh = apool2.tile([128, 4, H * E * BL], BF, tag="vh")
            for lc4 in range(4):
                lc = j * 4 + lc4
                for b in range(BL):
                    ps_v = aps.tile([128, 512], F32, tag="vps")
                    nc.tensor.matmul(ps_v[:], G2s[:, 0, lc, :], OT4[:, 0, :, :, b],
                                     start=True, stop=False)
                    nc.tensor.matmul(ps_v[:], G2s[:, 1, lc, :], OT4[:, 1, :, :, b],
                                     start=False, stop=True)
                    nc.scalar.activation(Vh[:, lc4, b * 512:(b + 1) * 512],
                                         ps_v[:], ACTF.Copy, bias=0.0)
            for b in range(BL):
                for n in range(DCH):
                    ps_o = aps.tile([128, 512], F32, tag="ops")
                    for k in range(DCH):
                        nc.tensor.matmul(ps_o[:], ows[:, k, n * 128:(n + 1) * 128],
                                         Vh[:, k, b * 512:(b + 1) * 512],
                                         start=(k == 0), stop=(k == DCH - 1))
                    xv = xT[:, n, b * L:(b + 1) * L].rearrange(
                        "p (h o two) -> p h o two", h=H, o=E, two=2)[:, :, :, j]
                    nc.vector.scalar_tensor_tensor(
                        out=xv, in0=ps_o[:].rearrange("p (h o) -> p h o", h=H),
                        scalar=1.0, in1=xv, op0=ALU.mult, op1=ALU.add,
                        accum_out=sl_z[:, n, b, li * 2 + j:li * 2 + j + 1])
        aps_cm.__exit__(None, None, None)

    # ============ decomp 1 ============
    _decomp(nc, tc, li, xT, sl_x, slot=li * 2 + 0)

    # ============ FFN ============
    with tc.tile_pool(name=f"ffn{li}", bufs=2) as fpool, \
         tc.tile_pool(name=f"ffnps{li}", bufs=2, space="PSUM") as fps, \
         tc.tile_pool(name=f"ffnpsg{li}", bufs=4, space="PSUM") as fpsg:
        for s2 in range(8):
            ps_g = [fpsg.tile([128, 512], F32, tag="gps", name=f"psg{n}")
                    for n in range(DCH)]
            for f in range(4):
                y1 = fpool.tile([128, 4, 512], BF, tag="y1")
                for mm in range(4):
                    ps_f = fps.tile([128, 512], F32, tag="fps")
                    for k in range(DCH):
                        nc.tensor.matmul(
                            ps_f[:], c1s[:, k, f * 512 + mm * 128:f * 512 + (mm + 1) * 128],
                            xT[:, k, s2 * 512:(s2 + 1) * 512],
                            start=(k == 0), stop=(k == DCH - 1))
                    nc.scalar.activation(y1[:, mm, :], ps_f[:], ACTF.Gelu, bias=0.0)
                for n in range(DCH):
                    for mm in range(4):
                        nc.tensor.matmul(ps_g[n][:],
                                         c2s[:, f * 4 + mm, n * 128:(n + 1) * 128],
                                         y1[:, mm, :],
                                         start=(f == 0 and mm == 0),
                                         stop=(f == 3 and mm == 3))
            b, half = s2 // 2, s2 % 2
            zslot = 4 + li * 2 + half
            for n in range(DCH):
                xsl = xT[:, n, s2 * 512:(s2 + 1) * 512]
                nc.vector.scalar_tensor_tensor(
                    out=xsl, in0=ps_g[n][:], scalar=1.0, in1=xsl,
                    op0=ALU.mult, op1=ALU.add,
                    accum_out=sl_z[:, n, b, zslot:zslot + 1])

    # ============ decomp 2 ============
    _decomp(nc, tc, li, xT, sl_x, slot=li * 2 + 1)


def _decomp(nc, tc, li, xT, sl_x, slot):
    with tc.tile_pool(name=f"dc{li}_{slot}", bufs=2) as dpool:
        for cch in range(DCH):
            for b in range(BL):
                stripe = dpool.tile([128, SW], BF, tag="stripe")
                nc.gpsimd.tensor_copy(stripe[:, PAD + 1:PAD + 1 + L],
                                      xT[:, cch, b * L:(b + 1) * L])
                nc.vector.tensor_copy(
                    stripe[:, 0:PAD + 1],
                    xT[:, cch, b * L:b * L + 1].to_broadcast((128, PAD + 1)))
                nc.vector.tensor_copy(
                    stripe[:, PAD + 1 + L:SW],
                    xT[:, cch, (b + 1) * L - 1:(b + 1) * L].to_broadcast((128, PAD)))
                init = dpool.tile([128, 1], F32, tag="init")
                nc.vector.reduce_sum(init[:], stripe[:, 0:K_MA], axis=AX.X)
                scano = dpool.tile([128, L], F32, tag="scano")
                nc.vector.tensor_tensor_scan(
                    scano[:], stripe[:, K_MA:SW], stripe[:, 0:L], init[:],
                    op0=ALU.add, op1=ALU.subtract)
                nc.vector.scalar_tensor_tensor(
                    out=xT[:, cch, b * L:(b + 1) * L], in0=scano[:],
                    scalar=-1.0 / K_MA, in1=xT[:, cch, b * L:(b + 1) * L],
                    op0=ALU.mult, op1=ALU.add,
                    accum_out=sl_x[:, cch, b, slot:slot + 1])


# ---------------------------------------------------------------------------
# entry point
# ---------------------------------------------------------------------------

_CACHE = {}


def kernel(x_enc, params, freq_index):
    consts = build_consts(params, freq_index)
    core_maps = stage_core_inputs(x_enc)
    if 'nc' not in _CACHE:
        _CACHE['nc'] = build_program()
    nc = _CACHE['nc']
    in_maps = [{**consts, **cm} for cm in core_maps]
    from concourse.bass_utils import run_bass_kernel_spmd
    res = run_bass_kernel_spmd(nc, in_maps, core_ids=list(range(NCORES)))
    out = np.concatenate([res.results[i]['out'] for i in range(NCORES)], axis=0)
    return out.astype(np.float32)


# revision 17
# speedup vs baseline: 1.2394x; 1.2394x over previous
"""DecompFEDformerEncoder Trainium2 kernel.

Data-parallel over batch (B=32 -> 4 per core x 8 cores), full model per core.
Residual stream x kept in [D(part-chunks), T] layout, bf16 master + fp32 accums.
Moving-average decomposition via tensor_tensor_scan sliding window.
Fourier block: DFT-as-matmul (64 low modes), per-(h,m) complex einsum as
weight-stationary [128x128] matmuls (complex folded into K), PE transposes,
iDFT-as-matmul. Trend stream never materialized: mean-over-L harvested from
fused accum_out on eviction ops via the telescoping identity
  enc = mean(x_final) + sum_j (mean z_j - mean x_j) + (MA0-mean @ tw + biases).
"""
import sys, os

for _p in ('/opt/trn_rl_repo', '/root/.axon_site/_ro/trn_rl_repo'):
    if os.path.isdir(_p) and _p not in sys.path:
        sys.path.insert(0, _p)

import numpy as np
import ml_dtypes

import concourse.bass as bass
import concourse.mybir as mybir
import concourse.tile as tile
from concourse import bacc

BF16 = ml_dtypes.bfloat16
F32 = mybir.dt.float32
BF = mybir.dt.bfloat16
ALU = mybir.AluOpType
ACTF = mybir.ActivationFunctionType
AX = mybir.AxisListType

# problem dims
B, L, FEAT = 32, 1024, 128
D, DFF, H, NLAYERS = 512, 2048, 8, 2
MODES, K_MA, CLS = 64, 25, 128
E = D // H               # 64
NCORES = 8
BL = B // NCORES         # 4 batches per core
T = BL * L               # 4096 tokens per core
PAD = (K_MA - 1) // 2    # 12
SW = L + K_MA            # stripe width 1049
DCH = D // 128           # 4 chunks of channels
NPAIR = H * MODES        # 512 einsum pairs


# ---------------------------------------------------------------------------
# host-side constant staging
# ---------------------------------------------------------------------------

def _posemb():
    pos = np.arange(L, dtype=np.float64)[:, None]
    div = np.exp(np.arange(0, D, 2, dtype=np.float64) * (-np.log(10000.0) / D))
    pe = np.zeros((L, D), np.float64)
    pe[:, 0::2] = np.sin(pos * div)
    pe[:, 1::2] = np.cos(pos * div)
    return pe.astype(np.float32)


def build_consts(params, freq_index):
    """Return dict of replicated (same on all cores) dram input arrays."""
    g = lambda a: np.asarray(a)
    fi = np.asarray(freq_index).astype(np.int64)   # [64] mode indices
    c = {}

    # DFT matrix: F2[l, m] = cos(2 pi f_m l / L); F2[l, 64+m] = -sin(...)
    ll = np.arange(L, dtype=np.float64)[:, None]
    ang = 2.0 * np.pi * fi[None, :] * ll / L       # [L, 64]
    F2 = np.concatenate([np.cos(ang), -np.sin(ang)], axis=1)  # [1024, 128]
    # swizzle to [128, 8, 128]: F2h[p, k, m2] = F2[k*128+p, m2]
    F2h = F2.reshape(8, 128, 128).transpose(1, 0, 2).astype(BF16).copy()

    # iDFT: out[l] = sum_m s_m (Re[m] cos - Im[m] sin), s_m = (2 - [f_m==0])/L
    s = (2.0 - (fi == 0).astype(np.float64)) / L
    lr = np.arange(L, dtype=np.float64)[None, :]
    angi = 2.0 * np.pi * fi[:, None] * lr / L      # [64, L]
    G2 = np.stack([s[:, None] * np.cos(angi), -s[:, None] * np.sin(angi)], 1)
    # [64, 2, 1024] -> view [64, 2, 8, 128]
    c['G2'] = G2.reshape(64, 2, 8, 128).astype(BF16).copy()
    c['F2'] = F2h

    layers = params['layers']
    qw_h, ow_h, c1_h, c2_h, W2_h, qbc_h = [], [], [], [], [], []
    obsum = np.zeros(D, np.float64)
    for lp in layers:
        qw = g(lp['qw']).astype(np.float64)        # [512, 512]
        qb = g(lp['qb']).astype(np.float64)        # [512]
        ow = g(lp['ow']).astype(np.float64)
        ob = g(lp['ob']).astype(np.float64)
        fwre = g(lp['fw_re']).astype(np.float64)   # [8, 64, 64, 64] (h,i,o,m)
        fwim = g(lp['fw_im']).astype(np.float64)
        c1 = g(lp['c1']).astype(np.float64)        # [512, 2048]
        c2 = g(lp['c2']).astype(np.float64)        # [2048, 512]
        obsum += ob

        qw_h.append(qw.reshape(4, 128, 512).transpose(1, 0, 2).reshape(128, 4 * 512))
        ow_h.append(ow.reshape(4, 128, 512).transpose(1, 0, 2).reshape(128, 4 * 512))
        c1_h.append(c1.reshape(4, 128, DFF).transpose(1, 0, 2).reshape(128, 4 * DFF))
        c2_h.append(c2.reshape(16, 128, 512).transpose(1, 0, 2).reshape(128, 16 * 512))

        # einsum stationary: for pair p=(h*64+m): Wt[kappa=(r_in*64+i), (r_out*64+o)]
        # [[re,  im],
        #  [-im, re]]
        Wt = np.zeros((NPAIR, 128, 128), np.float64)
        fre = fwre.transpose(0, 3, 1, 2)           # [h, m, i, o]
        fim = fwim.transpose(0, 3, 1, 2)
        fre = fre.reshape(NPAIR, 64, 64)
        fim = fim.reshape(NPAIR, 64, 64)
        Wt[:, 0:64, 0:64] = fre
        Wt[:, 0:64, 64:128] = fim
        Wt[:, 64:128, 0:64] = -fim
        Wt[:, 64:128, 64:128] = fre
        # swizzle [32 blocks, 128(kappa), 16, 128]: W2h[bl, p, j, o2]=Wt[bl*16+j, p, o2]
        W2_h.append(Wt.reshape(32, 16, 128, 128).transpose(0, 2, 1, 3)
                      .reshape(32, 128, 16 * 128))

        # q-bias correction on X: modes with f_m == 0 get += L * qb (real part)
        qbc = np.zeros((128, H), np.float64)
        qbc[0:64, :] = (L * qb).reshape(H, E).T    # rows (re, i), col h
        qbc_h.append(qbc)

    c['qw'] = np.stack(qw_h).astype(BF16)
    c['ow'] = np.stack(ow_h).astype(BF16)
    c['c1'] = np.stack(c1_h).astype(BF16)
    c['c2'] = np.stack(c2_h).astype(BF16)
    c['W2'] = np.stack(W2_h).astype(BF16)          # [2, 32, 128, 2048]
    c['qbc'] = np.stack(qbc_h).astype(BF16)        # [2, 128, 8]

    sw = g(params['seas_w']).astype(np.float64)    # [128, 512]
    sb = g(params['seas_b']).astype(np.float64)    # [512]
    tw = g(params['trend_w']).astype(np.float64)
    tb = g(params['trend_b']).astype(np.float64)
    cw = g(params['cls_w']).astype(np.float64)     # [512, 128]
    cb = g(params['cls_b']).astype(np.float64)     # [128]

    c['sw'] = sw.astype(BF16)                      # [128, 512] lhsT natural
    c['sbb'] = sb.reshape(4, 128).T.astype(np.float32).copy()   # [128, 4]
    c['tw'] = tw.astype(np.float32)                # [128, 512]
    c['tbias'] = (tb + obsum).reshape(4, 128).T.astype(np.float32).copy()
    c['clsw'] = cw.reshape(4, 128, CLS).transpose(1, 0, 2).reshape(128, 4 * CLS).astype(np.float32).copy()
    c['clsb'] = np.broadcast_to(cb.astype(np.float32), (BL, CLS)).copy()
    c['pos'] = _posemb().T.reshape(4, 128, L).transpose(1, 0, 2).reshape(128, 4 * L).astype(BF16).copy()
    c['identb'] = np.eye(128, dtype=BF16)
    return c


def stage_core_inputs(x_enc):
    """Per-core input staging: x in [feat, T] bf16 + fp32 MA stripes."""
    maps = []
    for ci in range(NCORES):
        xs = np.asarray(x_enc[ci * BL:(ci + 1) * BL]).astype(np.float32)  # [4,1024,128]
        xT = xs.reshape(T, FEAT).T.copy()                  # [128, 4096]
        stripes = np.zeros((FEAT, BL, SW), np.float32)
        for b in range(BL):
            xb = xs[b].T                                    # [128, 1024]
            stripes[:, b, PAD + 1:PAD + 1 + L] = xb
            stripes[:, b, 0:PAD + 1] = xb[:, 0:1]
            stripes[:, b, PAD + 1 + L:] = xb[:, -1:]
        maps.append({'x_encT': xT.astype(BF16),
                     'stripes0': stripes.reshape(FEAT, BL * SW)})
    return maps


# ---------------------------------------------------------------------------
# device program
# ---------------------------------------------------------------------------

def build_program():
    nc = bacc.Bacc("TRN2", target_bir_lowering=False, debug=False)

    dram = {}
    def din(name, shape, dt):
        dram[name] = nc.dram_tensor(name, list(shape), dt, kind="ExternalInput")
        return dram[name]

    x_encT_d = din('x_encT', [FEAT, T], BF)
    stripes0_d = din('stripes0', [FEAT, BL * SW], F32)
    F2_d = din('F2', [128, 8, 128], BF)
    G2_d = din('G2', [64, 2, 8, 128], BF)
    qw_d = din('qw', [NLAYERS, 128, 4 * 512], BF)
    ow_d = din('ow', [NLAYERS, 128, 4 * 512], BF)
    c1_d = din('c1', [NLAYERS, 128, 4 * DFF], BF)
    c2_d = din('c2', [NLAYERS, 128, 16 * 512], BF)
    W2_d = din('W2', [NLAYERS, 32, 128, 16 * 128], BF)
    qbc_d = din('qbc', [NLAYERS, 128, H], BF)
    sw_d = din('sw', [128, 512], BF)
    sbb_d = din('sbb', [128, 4], F32)
    tw_d = din('tw', [128, 512], F32)
    tbias_d = din('tbias', [128, 4], F32)
    clsw_d = din('clsw', [128, 4 * CLS], F32)
    clsb_d = din('clsb', [BL, CLS], F32)
    pos_d = din('pos', [128, 4 * L], BF)
    identb_d = din('identb', [128, 128], BF)
    out_d = nc.dram_tensor('out', [BL, CLS], F32, kind="ExternalOutput")

    with tile.TileContext(nc) as tc:
        # ------- persistent pools
        with tc.tile_pool(name="resid", bufs=1) as resid_pool, \
             tc.tile_pool(name="consts", bufs=1) as cpool, \
             tc.tile_pool(name="slots", bufs=1) as spool, \
             tc.tile_pool(name="wpool", bufs=1) as wpool:

            xT = resid_pool.tile([128, DCH, T], BF)        # residual stream
            F2s = cpool.tile([128, 8, 128], BF)
            nc.sync.dma_start(F2s[:], F2_d[:])
            G2s = cpool.tile([64, 2, 8, 128], BF)
            nc.sync.dma_start(G2s[:], G2_d[:])
            sws = cpool.tile([128, 512], BF)
            nc.sync.dma_start(sws[:], sw_d[:])
            sbbs = cpool.tile([128, 4], F32)
            nc.sync.dma_start(sbbs[:], sbb_d[:])
            tws = cpool.tile([128, 512], F32)
            nc.sync.dma_start(tws[:], tw_d[:])
            tbs = cpool.tile([128, 4], F32)
            nc.sync.dma_start(tbs[:], tbias_d[:])
            clsws = cpool.tile([128, 4 * CLS], F32)
            nc.sync.dma_start(clsws[:], clsw_d[:])
            clsbs = cpool.tile([BL, CLS], F32)
            nc.sync.dma_start(clsbs[:], clsb_d[:])
            identb = cpool.tile([128, 128], BF)
            nc.sync.dma_start(identb[:], identb_d[:])

            # accumulation slots (all fp32, written exactly once each)
            # z-slots per (c,b): 0..3 attn(li*2+j), 4..7 ffn(li*2+half), 8 x0
            sl_z = spool.tile([128, DCH, BL, 9], F32)
            sl_x = spool.tile([128, DCH, BL, 4], F32)
            sl_seas0 = spool.tile([128, BL], F32)      # feat-space
            sl_xenc = spool.tile([128, BL], F32)       # feat-space

            # ---------------- stage 0: initial decomposition + embedding
            with tc.tile_pool(name="emb", bufs=1) as epool, \
                 tc.tile_pool(name="emb2", bufs=2) as epool2, \
                 tc.tile_pool(name="embps", bufs=2, space="PSUM") as eps:
                xe = epool.tile([128, T], BF)
                nc.sync.dma_start(xe[:], x_encT_d[:])
                str0 = epool.tile([128, BL, SW], F32)
                nc.sync.dma_start(str0[:], stripes0_d[:].rearrange(
                    "p (b s) -> p b s", b=BL))
                seas0 = epool.tile([128, T], BF)

                # sum_l x_enc per (feat, b)
                nc.vector.reduce_sum(sl_xenc[:], xe[:].rearrange(
                    "p (b l) -> p b l", b=BL), axis=AX.X)

                for b in range(BL):
                    init = epool2.tile([128, 1], F32, tag="init")
                    nc.vector.reduce_sum(init[:], str0[:, b, 0:K_MA], axis=AX.X)
                    scano = epool2.tile([128, L], F32, tag="scano")
                    nc.vector.tensor_tensor_scan(
                        scano[:], str0[:, b, K_MA:SW], str0[:, b, 0:L], init[:],
                        op0=ALU.add, op1=ALU.subtract)
                    # seas0 = x_enc - scano/25 ; accum = sum(seas0)
                    nc.vector.scalar_tensor_tensor(
                        out=seas0[:, b * L:(b + 1) * L], in0=scano[:],
                        scalar=-1.0 / K_MA, in1=xe[:, b * L:(b + 1) * L],
                        op0=ALU.mult, op1=ALU.add, accum_out=sl_seas0[:, b:b + 1])

                # embedding: xT[n-chunk] = sw.T @ seas0 + sb ; then += pos
                for n in range(DCH):
                    for s2 in range(8):
                        ps_t = eps.tile([128, 512], F32, tag="embps")
                        nc.tensor.matmul(ps_t[:], sws[:, n * 128:(n + 1) * 128],
                                         seas0[:, s2 * 512:(s2 + 1) * 512],
                                         start=True, stop=True)
                        nc.scalar.activation(xT[:, n, s2 * 512:(s2 + 1) * 512],
                                             ps_t[:], ACTF.Identity,
                                             bias=sbbs[:, n:n + 1])
                for n in range(DCH):
                    posc = epool2.tile([128, L], BF, tag="pos")
                    nc.sync.dma_start(posc[:], pos_d[:, n * L:(n + 1) * L])
                    for b in range(BL):
                        nc.vector.scalar_tensor_tensor(
                            out=xT[:, n, b * L:(b + 1) * L], in0=posc[:],
                            scalar=1.0, in1=xT[:, n, b * L:(b + 1) * L],
                            op0=ALU.mult, op1=ALU.add,
                            accum_out=sl_z[:, n, b, 8:9])   # slot: sum(x0)

            # ---------------- layers
            for li in range(NLAYERS):
                qws = wpool.tile([128, 4, 512], BF, tag="qw")
                nc.sync.dma_start(qws[:], qw_d[li].rearrange("p (k n) -> p k n", k=4))
                ows = wpool.tile([128, 4, 512], BF, tag="ow")
                nc.sync.dma_start(ows[:], ow_d[li].rearrange("p (k n) -> p k n", k=4))
                c1s = wpool.tile([128, 4, DFF], BF, tag="c1")
                nc.sync.dma_start(c1s[:], c1_d[li].rearrange("p (k n) -> p k n", k=4))
                c2s = wpool.tile([128, 16, 512], BF, tag="c2")
                nc.sync.dma_start(c2s[:], c2_d[li].rearrange("p (k n) -> p k n", k=16))
                qbcs = wpool.tile([128, H], BF, tag="qbc")
                nc.sync.dma_start(qbcs[:], qbc_d[li])

                _layer(nc, tc, li, xT, F2s, G2s, identb, qws, ows, c1s, c2s,
                       qbcs, W2_d, sl_z, sl_x)

            # ---------------- final: combine means, classifier
            with tc.tile_pool(name="fin", bufs=1) as fpool, \
                 tc.tile_pool(name="finps", bufs=2, space="PSUM") as fps:
                zsum = fpool.tile([128, DCH, BL], F32)
                nc.vector.reduce_sum(zsum[:], sl_z[:, :, :, 0:8], axis=AX.X)
                xsum = fpool.tile([128, DCH, BL], F32)
                nc.vector.reduce_sum(xsum[:], sl_x[:, :, :, 0:3], axis=AX.X)
                acc = fpool.tile([128, DCH, BL], F32)
                nc.vector.tensor_tensor(acc[:], zsum[:], xsum[:], op=ALU.subtract)

                # meanMA0 = (sum x_enc - sum seas0)/L  [feat, b]
                mm0 = fpool.tile([128, BL], F32)
                nc.vector.tensor_tensor(mm0[:], sl_xenc[:], sl_seas0[:], op=ALU.subtract)
                nc.vector.tensor_scalar_mul(mm0[:], mm0[:], 1.0 / L)

                enc = fpool.tile([128, DCH, BL], F32)
                for cch in range(DCH):
                    ps_tw = fps.tile([128, BL], F32, tag="twps")
                    nc.tensor.matmul(ps_tw[:], tws[:, cch * 128:(cch + 1) * 128],
                                     mm0[:], start=True, stop=True)
                    # enc = acc/L + tw-term
                    nc.vector.scalar_tensor_tensor(
                        out=enc[:, cch, :], in0=acc[:, cch, :], scalar=1.0 / L,
                        in1=ps_tw[:], op0=ALU.mult, op1=ALU.add)
                    nc.vector.tensor_scalar(
                        enc[:, cch, :], enc[:, cch, :], tbs[:, cch:cch + 1], None,
                        op0=ALU.add)

                ps_cls = fps.tile([BL, CLS], F32, tag="clsps")
                for cch in range(DCH):
                    nc.tensor.matmul(ps_cls[:], enc[:, cch, :],
                                     clsws[:, cch * CLS:(cch + 1) * CLS],
                                     start=(cch == 0), stop=(cch == DCH - 1))
                outs = fpool.tile([BL, CLS], F32)
                nc.vector.tensor_tensor(outs[:], ps_cls[:], clsbs[:], op=ALU.add)
                nc.sync.dma_start(out_d[:], outs[:])

    nc.compile()
    return nc


def _layer(nc, tc, li, xT, F2s, G2s, identb, qws, ows, c1s, c2s, qbcs,
           W2_d, sl_z, sl_x):
    """Emit one encoder layer."""
    # ============ fourier attention ============
    with tc.tile_pool(name=f"att{li}", bufs=1) as apool, \
         tc.tile_pool(name=f"att2{li}", bufs=2) as apool2:

        Xstk = apool.tile([128, NPAIR * BL], BF)          # [(r,i), (h,m,b)]
        X4 = Xstk[:].rearrange("p (h m b) -> p h m b", h=H, m=MODES, b=BL)

        # ---- q projection + DFT, per batch
        aps_cm = tc.tile_pool(name=f"attps{li}", bufs=2, space="PSUM")
        aps = aps_cm.__enter__()
        for b in range(BL):
            qsb = apool2.tile([128, 8, 512], BF, tag="qsb")
            for lc in range(8):
                tg = (b * 8 + lc) * 128
                ps_q = aps.tile([128, 512], F32, tag="qps")
                for k in range(DCH):
                    nc.tensor.matmul(ps_q[:], xT[:, k, tg:tg + 128], qws[:, k, :],
                                     start=(k == 0), stop=(k == DCH - 1))
                nc.scalar.activation(qsb[:, lc, :], ps_q[:], ACTF.Copy, bias=0.0)
            for cch in range(DCH):
                ps_x = aps.tile([128, 128], F32, tag="xps")
                for lc in range(8):
                    nc.tensor.matmul(ps_x[:], qsb[:, lc, cch * 128:(cch + 1) * 128],
                                     F2s[:, lc, :], start=(lc == 0), stop=(lc == 7))
                h0 = 2 * cch
                # re rows (0:64) <- psum cols 0:64 ; im rows (64:128) <- cols 64:128
                nc.vector.tensor_copy(X4[0:64, h0, :, b], ps_x[0:64, 0:64])
                nc.vector.tensor_copy(X4[0:64, h0 + 1, :, b], ps_x[64:128, 0:64])
                nc.vector.tensor_copy(X4[64:128, h0, :, b], ps_x[0:64, 64:128])
                nc.vector.tensor_copy(X4[64:128, h0 + 1, :, b], ps_x[64:128, 64:128])

        # q-bias correction on zero-frequency modes (mode list baked in host F2;
        # correction column m=0 matches freq_index arange convention)
        nc.vector.tensor_tensor(X4[:, :, 0, :], X4[:, :, 0, :],
                                qbcs[:, :, None].to_broadcast((128, H, BL)),
                                op=ALU.add)
        aps_cm.__exit__(None, None, None)

        # ---- einsum: 512 weight-stationary matmuls
        aps_cm = tc.tile_pool(name=f"attps{li}e", bufs=2, space="PSUM")
        aps = aps_cm.__enter__()
        OutSel = apool.tile([128, NPAIR * BL], BF)        # [(r,o), (pair,b)]
        for g4 in range(4):                                # psum bank groups
            ps_e = aps.tile([128, 512], F32, tag="eps")
            for bl in range(g4 * 8, (g4 + 1) * 8):         # 8 weight blocks of 16
                wsb = apool2.tile([128, 16 * 128], BF, tag="wsb")
                nc.sync.dma_start(wsb[:], W2_d[li, bl])
                for j in range(16):
                    p = bl * 16 + j
                    nc.tensor.matmul(ps_e[:, (p % 128) * 4:(p % 128) * 4 + 4],
                                     wsb[:, j * 128:(j + 1) * 128],
                                     Xstk[:, p * 4:(p + 1) * 4],
                                     start=True, stop=True)
            nc.vector.tensor_copy(OutSel[:, g4 * 512:(g4 + 1) * 512], ps_e[:])
        aps_cm.__exit__(None, None, None)

        # ---- transpose OutSel[(r,o),(h,m,b)] -> OutT[m,(r,h,o,b)]
        aps_cm = tc.tile_pool(name=f"attps{li}t", bufs=2, space="PSUM")
        aps = aps_cm.__enter__()
        OutT = apool.tile([64, 2 * H * E * BL], BF)
        OT4 = OutT[:].rearrange("p (r h o b) -> p r h o b", r=2, h=H, o=E, b=BL)
        OS4 = OutSel[:].rearrange("p (pr b) -> p pr b", b=BL)
        for hh in range(H):
            for b in range(BL):
                ps_t = aps.tile([64, 128], BF, tag="tps")
                nc.tensor.transpose(ps_t[:], OS4[:, hh * 64:(hh + 1) * 64, b],
                                    identb[:])
                nc.vector.tensor_copy(OT4[:, 0, hh, :, b], ps_t[:, 0:64])
                nc.vector.tensor_copy(OT4[:, 1, hh, :, b], ps_t[:, 64:128])
        aps_cm.__exit__(None, None, None)

        # ---- iDFT + o-projection + residual, per j-half
        aps_cm = tc.tile_pool(name=f"attps{li}v", bufs=2, space="PSUM")
        aps = aps_cm.__enter__()
        for j in range(2):
            Vh = apool2.tile([128, 4, H * E * BL], BF, tag="vh")
            for lc4 in range(4):
                lc = j * 4 + lc4
                for b in range(BL):
                    ps_v = aps.tile([128, 512], F32, tag="vps")
                    nc.tensor.matmul(ps_v[:], G2s[:, 0, lc, :], OT4[:, 0, :, :, b],
                                     start=True, stop=False)
                    nc.tensor.matmul(ps_v[:], G2s[:, 1, lc, :], OT4[:, 1, :, :, b],
                                     start=False, stop=True)
                    nc.scalar.activation(Vh[:, lc4, b * 512:(b + 1) * 512],
                                         ps_v[:], ACTF.Copy, bias=0.0)
            for b in range(BL):
                for n in range(DCH):
                    ps_o = aps.tile([128, 512], F32, tag="ops")
                    for k in range(DCH):
                        nc.tensor.matmul(ps_o[:], ows[:, k, n * 128:(n + 1) * 128],
                                         Vh[:, k, b * 512:(b + 1) * 512],
                                         start=(k == 0), stop=(k == DCH - 1))
                    xv = xT[:, n, b * L:(b + 1) * L].rearrange(
                        "p (h o two) -> p h o two", h=H, o=E, two=2)[:, :, :, j]
                    nc.vector.scalar_tensor_tensor(
                        out=xv, in0=ps_o[:].rearrange("p (h o) -> p h o", h=H),
                        scalar=1.0, in1=xv, op0=ALU.mult, op1=ALU.add,
                        accum_out=sl_z[:, n, b, li * 2 + j:li * 2 + j + 1])
        aps_cm.__exit__(None, None, None)

    # ============ decomp 1 ============
    _decomp(nc, tc, li, xT, sl_x, slot=li * 2 + 0)

    # ============ FFN ============
    with tc.tile_pool(name=f"ffn{li}", bufs=2) as fpool, \
         tc.tile_pool(name=f"ffnps{li}", bufs=2, space="PSUM") as fps, \
         tc.tile_pool(name=f"ffnpsg{li}", bufs=4, space="PSUM") as fpsg:
        for s2 in range(8):
            ps_g = [fpsg.tile([128, 512], F32, tag="gps", name=f"psg{n}")
                    for n in range(DCH)]
            for f in range(4):
                y1 = fpool.tile([128, 4, 512], BF, tag="y1")
                for mm in range(4):
                    ps_f = fps.tile([128, 512], F32, tag="fps")
                    for k in range(DCH):
                        nc.tensor.matmul(
                            ps_f[:], c1s[:, k, f * 512 + mm * 128:f * 512 + (mm + 1) * 128],
                            xT[:, k, s2 * 512:(s2 + 1) * 512],
                            start=(k == 0), stop=(k == DCH - 1))
                    nc.scalar.activation(y1[:, mm, :], ps_f[:], ACTF.Gelu, bias=0.0)
                for n in range(DCH):
                    for mm in range(4):
                        nc.tensor.matmul(ps_g[n][:],
                                         c2s[:, f * 4 + mm, n * 128:(n + 1) * 128],
                                         y1[:, mm, :],
                                         start=(f == 0 and mm == 0),
                                         stop=(f == 3 and mm == 3))
            b, half = s2 // 2, s2 % 2
            zslot = 4 + li * 2 + half
            for n in range(DCH):
                xsl = xT[:, n, s2 * 512:(s2 + 1) * 512]
                nc.vector.scalar_tensor_tensor(
                    out=xsl, in0=ps_g[n][:], scalar=1.0, in1=xsl,
                    op0=ALU.mult, op1=ALU.add,
                    accum_out=sl_z[:, n, b, zslot:zslot + 1])

    # ============ decomp 2 ============
    # (skipped for the last layer: the final seasonal x4 only enters the
    #  output via mean(x4) + mean(MA(z4)) = mean(z4), already accumulated
    #  at the FFN eviction)
    if li < NLAYERS - 1:
        _decomp(nc, tc, li, xT, sl_x, slot=li * 2 + 1)


def _decomp(nc, tc, li, xT, sl_x, slot):
    with tc.tile_pool(name=f"dc{li}_{slot}", bufs=2) as dpool:
        for b in range(BL):
            for cch in range(DCH):
                stripe = dpool.tile([128, SW], BF, tag="stripe")
                nc.scalar.copy(stripe[:, PAD + 1:PAD + 1 + L],
                               xT[:, cch, b * L:(b + 1) * L])
                nc.scalar.copy(
                    stripe[:, 0:PAD + 1],
                    xT[:, cch, b * L:b * L + 1].to_broadcast((128, PAD + 1)))
                nc.scalar.copy(
                    stripe[:, PAD + 1 + L:SW],
                    xT[:, cch, (b + 1) * L - 1:(b + 1) * L].to_broadcast((128, PAD)))
                init = dpool.tile([128, 1], F32, tag="init")
                nc.vector.reduce_sum(init[:], stripe[:, 0:K_MA], axis=AX.X)
                scano = dpool.tile([128, L], F32, tag="scano")
                nc.vector.tensor_tensor_scan(
                    scano[:], stripe[:, K_MA:SW], stripe[:, 0:L], init[:],
                    op0=ALU.add, op1=ALU.subtract)
                nc.vector.scalar_tensor_tensor(
                    out=xT[:, cch, b * L:(b + 1) * L], in0=scano[:],
                    scalar=-1.0 / K_MA, in1=xT[:, cch, b * L:(b + 1) * L],
                    op0=ALU.mult, op1=ALU.add,
                    accum_out=sl_x[:, cch, b, slot:slot + 1])


# ---------------------------------------------------------------------------
# entry point
# ---------------------------------------------------------------------------

_CACHE = {}


def kernel(x_enc, params, freq_index):
    consts = build_consts(params, freq_index)
    core_maps = stage_core_inputs(x_enc)
    if 'nc' not in _CACHE:
        _CACHE['nc'] = build_program()
    nc = _CACHE['nc']
    in_maps = [{**consts, **cm} for cm in core_maps]
    from concourse.bass_utils import run_bass_kernel_spmd
    res = run_bass_kernel_spmd(nc, in_maps, core_ids=list(range(NCORES)))
    out = np.concatenate([res.results[i]['out'] for i in range(NCORES)], axis=0)
    return out.astype(np.float32)


# revision 29
# speedup vs baseline: 1.3516x; 1.0905x over previous
"""DecompFEDformerEncoder Trainium2 kernel.

Data-parallel over batch (B=32 -> 4 per core x 8 cores), full model per core.
Residual stream x kept in [D(part-chunks), T] layout, bf16 master + fp32 accums.
Moving-average decomposition via tensor_tensor_scan sliding window.
Fourier block: DFT-as-matmul (64 low modes), per-(h,m) complex einsum as
weight-stationary [128x128] matmuls (complex folded into K), PE transposes,
iDFT-as-matmul. Trend stream never materialized: mean-over-L harvested from
fused accum_out on eviction ops via the telescoping identity
  enc = mean(x_final) + sum_j (mean z_j - mean x_j) + (MA0-mean @ tw + biases).
"""
import sys, os

for _p in ('/opt/trn_rl_repo', '/root/.axon_site/_ro/trn_rl_repo'):
    if os.path.isdir(_p) and _p not in sys.path:
        sys.path.insert(0, _p)

import numpy as np
import ml_dtypes

import concourse.bass as bass
import concourse.mybir as mybir
import concourse.tile as tile
from concourse import bacc

BF16 = ml_dtypes.bfloat16
F32 = mybir.dt.float32
BF = mybir.dt.bfloat16
ALU = mybir.AluOpType
ACTF = mybir.ActivationFunctionType
AX = mybir.AxisListType

# problem dims
B, L, FEAT = 32, 1024, 128
D, DFF, H, NLAYERS = 512, 2048, 8, 2
MODES, K_MA, CLS = 64, 25, 128
E = D // H               # 64
NCORES = 8
BL = B // NCORES         # 4 batches per core
T = BL * L               # 4096 tokens per core
PAD = (K_MA - 1) // 2    # 12
SW = L + K_MA            # stripe width 1049
DCH = D // 128           # 4 chunks of channels
NPAIR = H * MODES        # 512 einsum pairs


# ---------------------------------------------------------------------------
# host-side constant staging
# ---------------------------------------------------------------------------

def _posemb():
    pos = np.arange(L, dtype=np.float64)[:, None]
    div = np.exp(np.arange(0, D, 2, dtype=np.float64) * (-np.log(10000.0) / D))
    pe = np.zeros((L, D), np.float64)
    pe[:, 0::2] = np.sin(pos * div)
    pe[:, 1::2] = np.cos(pos * div)
    return pe.astype(np.float32)


def build_consts(params, freq_index):
    """Return dict of replicated (same on all cores) dram input arrays."""
    g = lambda a: np.asarray(a)
    fi = np.asarray(freq_index).astype(np.int64)   # [64] mode indices
    c = {}

    # DFT matrix: F2[l, m] = cos(2 pi f_m l / L); F2[l, 64+m] = -sin(...)
    ll = np.arange(L, dtype=np.float64)[:, None]
    ang = 2.0 * np.pi * fi[None, :] * ll / L       # [L, 64]
    F2 = np.concatenate([np.cos(ang), -np.sin(ang)], axis=1)  # [1024, 128]
    # swizzle to [128, 8, 128]: F2h[p, k, m2] = F2[k*128+p, m2]
    F2h = F2.reshape(8, 128, 128).transpose(1, 0, 2).astype(BF16).copy()

    # iDFT: out[l] = sum_m s_m (Re[m] cos - Im[m] sin), s_m = (2 - [f_m==0])/L
    s = (2.0 - (fi == 0).astype(np.float64)) / L
    lr = np.arange(L, dtype=np.float64)[None, :]
    angi = 2.0 * np.pi * fi[:, None] * lr / L      # [64, L]
    G2big = np.concatenate([s[:, None] * np.cos(angi),
                            -s[:, None] * np.sin(angi)], 0)  # [128=(r,m), L]
    c['F2'] = F2h

    layers = params['layers']
    qw_h, M2_h, c1_h, c2_h, W2_h, qbc_h = [], [], [], [], [], []
    obsum = np.zeros(D, np.float64)
    for lp in layers:
        qw = g(lp['qw']).astype(np.float64)        # [512, 512]
        qb = g(lp['qb']).astype(np.float64)        # [512]
        ow = g(lp['ow']).astype(np.float64)
        ob = g(lp['ob']).astype(np.float64)
        fwre = g(lp['fw_re']).astype(np.float64)   # [8, 64, 64, 64] (h,i,o,m)
        fwim = g(lp['fw_im']).astype(np.float64)
        c1 = g(lp['c1']).astype(np.float64)        # [512, 2048]
        c2 = g(lp['c2']).astype(np.float64)        # [2048, 512]
        obsum += ob

        qw_h.append(qw.reshape(4, 128, 512).transpose(1, 0, 2).reshape(128, 4 * 512))
        # fused iDFT + o-proj: M2[j][(r,m), n] = sum_d' G2big[(r,m), 512j+d'] ow[d', n]
        M2_h.append(np.stack([G2big[:, 512 * j:512 * (j + 1)] @ ow
                              for j in range(2)]))          # [2, 128, 512]
        c1_h.append(c1.reshape(4, 128, DFF).transpose(1, 0, 2).reshape(128, 4 * DFF))
        c2_h.append(c2.reshape(16, 128, 512).transpose(1, 0, 2).reshape(128, 16 * 512))

        # einsum stationary: for pair p=(h*64+m): Wt[kappa=(r_in*64+i), (r_out*64+o)]
        # [[re,  im],
        #  [-im, re]]
        Wt = np.zeros((NPAIR, 128, 128), np.float64)
        fre = fwre.transpose(0, 3, 1, 2)           # [h, m, i, o]
        fim = fwim.transpose(0, 3, 1, 2)
        fre = fre.reshape(NPAIR, 64, 64)
        fim = fim.reshape(NPAIR, 64, 64)
        Wt[:, 0:64, 0:64] = fre
        Wt[:, 0:64, 64:128] = fim
        Wt[:, 64:128, 0:64] = -fim
        Wt[:, 64:128, 64:128] = fre
        # swizzle [32 blocks, 128(kappa), 16, 128]: W2h[bl, p, j, o2]=Wt[bl*16+j, p, o2]
        W2_h.append(Wt.reshape(32, 16, 128, 128).transpose(0, 2, 1, 3)
                      .reshape(32, 128, 16 * 128))

        # q-bias correction on X: modes with f_m == 0 get += L * qb (real part)
        qbc = np.zeros((128, H), np.float64)
        qbc[0:64, :] = (L * qb).reshape(H, E).T    # rows (re, i), col h
        qbc_h.append(qbc)

    c['qw'] = np.stack(qw_h).astype(BF16)
    c['M2'] = np.stack(M2_h).astype(BF16)          # [2, 2, 128, 512]
    c['c1'] = np.stack(c1_h).astype(BF16)
    c['c2'] = np.stack(c2_h).astype(BF16)
    c['W2'] = np.stack(W2_h).astype(BF16)          # [2, 32, 128, 2048]
    c['qbc'] = np.stack(qbc_h).astype(BF16)        # [2, 128, 8]

    sw = g(params['seas_w']).astype(np.float64)    # [128, 512]
    sb = g(params['seas_b']).astype(np.float64)    # [512]
    tw = g(params['trend_w']).astype(np.float64)
    tb = g(params['trend_b']).astype(np.float64)
    cw = g(params['cls_w']).astype(np.float64)     # [512, 128]
    cb = g(params['cls_b']).astype(np.float64)     # [128]

    c['sw'] = sw.astype(BF16)                      # [128, 512] lhsT natural
    c['sbb'] = sb.reshape(4, 128).T.astype(np.float32).copy()   # [128, 4]
    c['tw'] = tw.astype(np.float32)                # [128, 512]
    c['tbias'] = (tb + obsum).reshape(4, 128).T.astype(np.float32).copy()
    c['clsw'] = cw.reshape(4, 128, CLS).transpose(1, 0, 2).reshape(128, 4 * CLS).astype(np.float32).copy()
    c['clsb'] = np.broadcast_to(cb.astype(np.float32), (BL, CLS)).copy()
    c['pos'] = _posemb().T.reshape(4, 128, L).transpose(1, 0, 2).reshape(128, 4 * L).astype(BF16).copy()
    c['identb'] = np.eye(128, dtype=BF16)
    return c


def stage_core_inputs(x_enc):
    """Per-core input staging: x in [feat, T] bf16 + fp32 MA stripes."""
    maps = []
    for ci in range(NCORES):
        xs = np.asarray(x_enc[ci * BL:(ci + 1) * BL]).astype(np.float32)  # [4,1024,128]
        xT = xs.reshape(T, FEAT).T.copy()                  # [128, 4096]
        stripes = np.zeros((FEAT, BL, SW), np.float32)
        for b in range(BL):
            xb = xs[b].T                                    # [128, 1024]
            stripes[:, b, PAD + 1:PAD + 1 + L] = xb
            stripes[:, b, 0:PAD + 1] = xb[:, 0:1]
            stripes[:, b, PAD + 1 + L:] = xb[:, -1:]
        maps.append({'x_encT': xT.astype(BF16),
                     'stripes0': stripes.reshape(FEAT, BL * SW)})
    return maps


# ---------------------------------------------------------------------------
# device program
# ---------------------------------------------------------------------------

def build_program():
    nc = bacc.Bacc("TRN2", target_bir_lowering=False, debug=False)

    dram = {}
    def din(name, shape, dt):
        dram[name] = nc.dram_tensor(name, list(shape), dt, kind="ExternalInput")
        return dram[name]

    x_encT_d = din('x_encT', [FEAT, T], BF)
    stripes0_d = din('stripes0', [FEAT, BL * SW], F32)
    F2_d = din('F2', [128, 8, 128], BF)
    qw_d = din('qw', [NLAYERS, 128, 4 * 512], BF)
    M2_d = din('M2', [NLAYERS, 2, 128, 512], BF)
    c1_d = din('c1', [NLAYERS, 128, 4 * DFF], BF)
    c2_d = din('c2', [NLAYERS, 128, 16 * 512], BF)
    W2_d = din('W2', [NLAYERS, 32, 128, 16 * 128], BF)
    qbc_d = din('qbc', [NLAYERS, 128, H], BF)
    sw_d = din('sw', [128, 512], BF)
    sbb_d = din('sbb', [128, 4], F32)
    tw_d = din('tw', [128, 512], F32)
    tbias_d = din('tbias', [128, 4], F32)
    clsw_d = din('clsw', [128, 4 * CLS], F32)
    clsb_d = din('clsb', [BL, CLS], F32)
    pos_d = din('pos', [128, 4 * L], BF)
    identb_d = din('identb', [128, 128], BF)
    out_d = nc.dram_tensor('out', [BL, CLS], F32, kind="ExternalOutput")

    with tile.TileContext(nc) as tc:
        # ------- persistent pools
        with tc.tile_pool(name="resid", bufs=1) as resid_pool, \
             tc.tile_pool(name="consts", bufs=1) as cpool, \
             tc.tile_pool(name="slots", bufs=1) as spool, \
             tc.tile_pool(name="wpool", bufs=1) as wpool:

            xT = resid_pool.tile([128, DCH, T], BF)        # residual stream
            F2s = cpool.tile([128, 8, 128], BF)
            nc.sync.dma_start(F2s[:], F2_d[:])
            sws = cpool.tile([128, 512], BF)
            nc.sync.dma_start(sws[:], sw_d[:])
            sbbs = cpool.tile([128, 4], F32)
            nc.sync.dma_start(sbbs[:], sbb_d[:])
            tws = cpool.tile([128, 512], F32)
            nc.sync.dma_start(tws[:], tw_d[:])
            tbs = cpool.tile([128, 4], F32)
            nc.sync.dma_start(tbs[:], tbias_d[:])
            clsws = cpool.tile([128, 4 * CLS], F32)
            nc.sync.dma_start(clsws[:], clsw_d[:])
            clsbs = cpool.tile([BL, CLS], F32)
            nc.sync.dma_start(clsbs[:], clsb_d[:])
            identb = cpool.tile([128, 128], BF)
            nc.sync.dma_start(identb[:], identb_d[:])

            # accumulation slots (all fp32, written exactly once each)
            # z-slots per (c,b): 0..3 attn(li*2+j), 4..7 ffn(li*2+half), 8 x0
            sl_z = spool.tile([128, DCH, BL, 9], F32)
            sl_x = spool.tile([128, DCH, BL, 4], F32)
            sl_seas0 = spool.tile([128, BL], F32)      # feat-space
            sl_xenc = spool.tile([128, BL], F32)       # feat-space

            # ---------------- stage 0: initial decomposition + embedding
            with tc.tile_pool(name="emb", bufs=1) as epool, \
                 tc.tile_pool(name="emb2", bufs=2) as epool2, \
                 tc.tile_pool(name="embps", bufs=2, space="PSUM") as eps:
                xe = epool.tile([128, T], BF)
                nc.sync.dma_start(xe[:], x_encT_d[:])
                str0 = epool.tile([128, BL, SW], F32)
                nc.sync.dma_start(str0[:], stripes0_d[:].rearrange(
                    "p (b s) -> p b s", b=BL))
                seas0 = epool.tile([128, T], BF)

                # sum_l x_enc per (feat, b)
                nc.vector.reduce_sum(sl_xenc[:], xe[:].rearrange(
                    "p (b l) -> p b l", b=BL), axis=AX.X)

                for b in range(BL):
                    init = epool2.tile([128, 1], F32, tag="init")
                    nc.vector.reduce_sum(init[:], str0[:, b, 0:K_MA], axis=AX.X)
                    scano = epool2.tile([128, L], F32, tag="scano")
                    nc.vector.tensor_tensor_scan(
                        scano[:], str0[:, b, K_MA:SW], str0[:, b, 0:L], init[:],
                        op0=ALU.add, op1=ALU.subtract)
                    # seas0 = x_enc - scano/25 ; accum = sum(seas0)
                    nc.vector.scalar_tensor_tensor(
                        out=seas0[:, b * L:(b + 1) * L], in0=scano[:],
                        scalar=-1.0 / K_MA, in1=xe[:, b * L:(b + 1) * L],
                        op0=ALU.mult, op1=ALU.add, accum_out=sl_seas0[:, b:b + 1])

                # embedding: xT[n-chunk] = sw.T @ seas0 + sb ; then += pos
                for n in range(DCH):
                    for s2 in range(8):
                        ps_t = eps.tile([128, 512], F32, tag="embps")
                        nc.tensor.matmul(ps_t[:], sws[:, n * 128:(n + 1) * 128],
                                         seas0[:, s2 * 512:(s2 + 1) * 512],
                                         start=True, stop=True)
                        nc.scalar.activation(xT[:, n, s2 * 512:(s2 + 1) * 512],
                                             ps_t[:], ACTF.Identity,
                                             bias=sbbs[:, n:n + 1])
                for n in range(DCH):
                    posc = epool2.tile([128, L], BF, tag="pos")
                    nc.sync.dma_start(posc[:], pos_d[:, n * L:(n + 1) * L])
                    for b in range(BL):
                        nc.vector.scalar_tensor_tensor(
                            out=xT[:, n, b * L:(b + 1) * L], in0=posc[:],
                            scalar=1.0, in1=xT[:, n, b * L:(b + 1) * L],
                            op0=ALU.mult, op1=ALU.add,
                            accum_out=sl_z[:, n, b, 8:9])   # slot: sum(x0)

            # ---------------- layers
            for li in range(NLAYERS):
                qws = wpool.tile([128, 4, 512], BF, tag="qw")
                nc.sync.dma_start(qws[:], qw_d[li].rearrange("p (k n) -> p k n", k=4))
                m2s = wpool.tile([128, 2, 512], BF, tag="m2")
                nc.sync.dma_start(m2s[:], M2_d[li])
                c1s = wpool.tile([128, 4, DFF], BF, tag="c1")
                nc.sync.dma_start(c1s[:], c1_d[li].rearrange("p (k n) -> p k n", k=4))
                c2s = wpool.tile([128, 16, 512], BF, tag="c2")
                nc.sync.dma_start(c2s[:], c2_d[li].rearrange("p (k n) -> p k n", k=16))
                qbcs = wpool.tile([128, H], BF, tag="qbc")
                nc.sync.dma_start(qbcs[:], qbc_d[li])

                _layer(nc, tc, li, xT, F2s, identb, qws, m2s, c1s, c2s,
                       qbcs, W2_d, sl_z, sl_x)

            # ---------------- final: combine means, classifier
            with tc.tile_pool(name="fin", bufs=1) as fpool, \
                 tc.tile_pool(name="finps", bufs=2, space="PSUM") as fps:
                zsum = fpool.tile([128, DCH, BL], F32)
                nc.vector.reduce_sum(zsum[:], sl_z[:, :, :, 0:8], axis=AX.X)
                xsum = fpool.tile([128, DCH, BL], F32)
                nc.vector.reduce_sum(xsum[:], sl_x[:, :, :, 0:3], axis=AX.X)
                acc = fpool.tile([128, DCH, BL], F32)
                nc.vector.tensor_tensor(acc[:], zsum[:], xsum[:], op=ALU.subtract)

                # meanMA0 = (sum x_enc - sum seas0)/L  [feat, b]
                mm0 = fpool.tile([128, BL], F32)
                nc.vector.tensor_tensor(mm0[:], sl_xenc[:], sl_seas0[:], op=ALU.subtract)
                nc.vector.tensor_scalar_mul(mm0[:], mm0[:], 1.0 / L)

                enc = fpool.tile([128, DCH, BL], F32)
                for cch in range(DCH):
                    ps_tw = fps.tile([128, BL], F32, tag="twps")
                    nc.tensor.matmul(ps_tw[:], tws[:, cch * 128:(cch + 1) * 128],
                                     mm0[:], start=True, stop=True)
                    # enc = acc/L + tw-term
                    nc.vector.scalar_tensor_tensor(
                        out=enc[:, cch, :], in0=acc[:, cch, :], scalar=1.0 / L,
                        in1=ps_tw[:], op0=ALU.mult, op1=ALU.add)
                    nc.vector.tensor_scalar(
                        enc[:, cch, :], enc[:, cch, :], tbs[:, cch:cch + 1], None,
                        op0=ALU.add)

                ps_cls = fps.tile([BL, CLS], F32, tag="clsps")
                for cch in range(DCH):
                    nc.tensor.matmul(ps_cls[:], enc[:, cch, :],
                                     clsws[:, cch * CLS:(cch + 1) * CLS],
                                     start=(cch == 0), stop=(cch == DCH - 1))
                outs = fpool.tile([BL, CLS], F32)
                nc.vector.tensor_tensor(outs[:], ps_cls[:], clsbs[:], op=ALU.add)
                nc.sync.dma_start(out_d[:], outs[:])

    nc.compile()
    return nc


def _layer(nc, tc, li, xT, F2s, identb, qws, m2s, c1s, c2s, qbcs,
           W2_d, sl_z, sl_x):
    """Emit one encoder layer."""
    # ============ fourier attention ============
    with tc.tile_pool(name=f"att{li}", bufs=1) as apool, \
         tc.tile_pool(name=f"att2{li}", bufs=2) as apool2:

        Xstk = apool.tile([128, NPAIR * BL], BF)          # [(r,i), (h,m,b)]
        X4 = Xstk[:].rearrange("p (h m b) -> p h m b", h=H, m=MODES, b=BL)

        # ---- q projection + DFT, per batch
        aps_cm = tc.tile_pool(name=f"attps{li}", bufs=2, space="PSUM")
        aps = aps_cm.__enter__()
        for b in range(BL):
            qsb = apool2.tile([128, 8, 512], BF, tag="qsb")
            for lc in range(8):
                tg = (b * 8 + lc) * 128
                ps_q = aps.tile([128, 512], F32, tag="qps")
                for k in range(DCH):
                    nc.tensor.matmul(ps_q[:], xT[:, k, tg:tg + 128], qws[:, k, :],
                                     start=(k == 0), stop=(k == DCH - 1))
                nc.scalar.activation(qsb[:, lc, :], ps_q[:], ACTF.Copy, bias=0.0)
            for cch in range(DCH):
                ps_x = aps.tile([128, 128], F32, tag="xps")
                for lc in range(8):
                    nc.tensor.matmul(ps_x[:], qsb[:, lc, cch * 128:(cch + 1) * 128],
                                     F2s[:, lc, :], start=(lc == 0), stop=(lc == 7))
                h0 = 2 * cch
                # re rows (0:64) <- psum cols 0:64 ; im rows (64:128) <- cols 64:128
                nc.vector.tensor_copy(X4[0:64, h0, :, b], ps_x[0:64, 0:64])
                nc.vector.tensor_copy(X4[0:64, h0 + 1, :, b], ps_x[64:128, 0:64])
                nc.vector.tensor_copy(X4[64:128, h0, :, b], ps_x[0:64, 64:128])
                nc.vector.tensor_copy(X4[64:128, h0 + 1, :, b], ps_x[64:128, 64:128])

        # q-bias correction on zero-frequency modes (mode list baked in host F2;
        # correction column m=0 matches freq_index arange convention)
        nc.vector.tensor_tensor(X4[:, :, 0, :], X4[:, :, 0, :],
                                qbcs[:, :, None].to_broadcast((128, H, BL)),
                                op=ALU.add)
        aps_cm.__exit__(None, None, None)

        # ---- einsum: 512 weight-stationary matmuls
        aps_cm = tc.tile_pool(name=f"attps{li}e", bufs=2, space="PSUM")
        aps = aps_cm.__enter__()
        apt_cm = tc.tile_pool(name=f"attps{li}t", bufs=2, space="PSUM")
        apt = apt_cm.__enter__()
        OutSel = apool.tile([128, NPAIR * BL], BF)        # [(r,o), (pair,b)]
        OutT = apool.tile([128, BL * H * E], BF)          # [(r,m), (b,h,o)]
        OT4 = OutT[:].rearrange("p (b h o) -> p b h o", b=BL, h=H)
        OS4 = OutSel[:].rearrange("p (pr b) -> p pr b", b=BL)
        for g4 in range(4):                                # psum bank groups
            ps_e = aps.tile([128, 512], F32, tag="eps")
            for bl in range(g4 * 8, (g4 + 1) * 8):         # 8 weight blocks of 16
                wsb = apool2.tile([128, 16 * 128], BF, tag="wsb")
                nc.sync.dma_start(wsb[:], W2_d[li, bl])
                for j in range(16):
                    p = bl * 16 + j
                    nc.tensor.matmul(ps_e[:, (p % 128) * 4:(p % 128) * 4 + 4],
                                     wsb[:, j * 128:(j + 1) * 128],
                                     Xstk[:, p * 4:(p + 1) * 4],
                                     start=True, stop=True)
            nc.vector.tensor_copy(OutSel[:, g4 * 512:(g4 + 1) * 512], ps_e[:])
            # transpose the two heads this group just produced into
            # OutT2[(r,m), (b, h, o)] (r via partition-offset evictions)
            for hh in (2 * g4, 2 * g4 + 1):
                for b in range(BL):
                    for r in range(2):
                        ps_t = apt.tile([64, 64], BF, tag="tps")
                        nc.tensor.transpose(
                            ps_t[:], OS4[r * 64:(r + 1) * 64,
                                         hh * 64:(hh + 1) * 64, b],
                            identb[r * 64:(r + 1) * 64, r * 64:(r + 1) * 64])
                        nc.vector.tensor_copy(OT4[r * 64:(r + 1) * 64, b, hh, :],
                                              ps_t[:])
        apt_cm.__exit__(None, None, None)
        aps_cm.__exit__(None, None, None)

        # ---- fused iDFT + o-projection + residual (attn.T = M2_j.T @ OutT2)
        aps_cm = tc.tile_pool(name=f"attps{li}v", bufs=2, space="PSUM")
        aps = aps_cm.__enter__()
        for b in range(BL):
            for j in range(2):
                for n in range(DCH):
                    ps_o = aps.tile([128, 512], F32, tag="ops")
                    nc.tensor.matmul(ps_o[:], m2s[:, j, n * 128:(n + 1) * 128],
                                     OutT[:, b * 512:(b + 1) * 512],
                                     start=True, stop=True)
                    xv = xT[:, n, b * L:(b + 1) * L].rearrange(
                        "p (h o two) -> p h o two", h=H, o=E, two=2)[:, :, :, j]
                    nc.vector.scalar_tensor_tensor(
                        out=xv, in0=ps_o[:].rearrange("p (h o) -> p h o", h=H),
                        scalar=1.0, in1=xv, op0=ALU.mult, op1=ALU.add,
                        accum_out=sl_z[:, n, b, li * 2 + j:li * 2 + j + 1])
        aps_cm.__exit__(None, None, None)

    # ============ decomp 1 ============
    _decomp(nc, tc, li, xT, sl_x, slot=li * 2 + 0)

    # ============ FFN ============
    with tc.tile_pool(name=f"ffn{li}", bufs=2) as fpool, \
         tc.tile_pool(name=f"ffnps{li}", bufs=2, space="PSUM") as fps, \
         tc.tile_pool(name=f"ffnpsg{li}", bufs=4, space="PSUM") as fpsg:
        for s2 in range(8):
            ps_g = [fpsg.tile([128, 512], F32, tag="gps", name=f"psg{n}")
                    for n in range(DCH)]
            for f in range(4):
                y1 = fpool.tile([128, 4, 512], BF, tag="y1")
                for mm in range(4):
                    ps_f = fps.tile([128, 512], F32, tag="fps")
                    for k in range(DCH):
                        nc.tensor.matmul(
                            ps_f[:], c1s[:, k, f * 512 + mm * 128:f * 512 + (mm + 1) * 128],
                            xT[:, k, s2 * 512:(s2 + 1) * 512],
                            start=(k == 0), stop=(k == DCH - 1))
                    nc.scalar.activation(y1[:, mm, :], ps_f[:], ACTF.Gelu, bias=0.0)
                for n in range(DCH):
                    for mm in range(4):
                        nc.tensor.matmul(ps_g[n][:],
                                         c2s[:, f * 4 + mm, n * 128:(n + 1) * 128],
                                         y1[:, mm, :],
                                         start=(f == 0 and mm == 0),
                                         stop=(f == 3 and mm == 3))
            b, half = s2 // 2, s2 % 2
            zslot = 4 + li * 2 + half
            for n in range(DCH):
                xsl = xT[:, n, s2 * 512:(s2 + 1) * 512]
                nc.vector.scalar_tensor_tensor(
                    out=xsl, in0=ps_g[n][:], scalar=1.0, in1=xsl,
                    op0=ALU.mult, op1=ALU.add,
                    accum_out=sl_z[:, n, b, zslot:zslot + 1])

    # ============ decomp 2 ============
    # (skipped for the last layer: the final seasonal x4 only enters the
    #  output via mean(x4) + mean(MA(z4)) = mean(z4), already accumulated
    #  at the FFN eviction)
    if li < NLAYERS - 1:
        _decomp(nc, tc, li, xT, sl_x, slot=li * 2 + 1)


def _decomp(nc, tc, li, xT, sl_x, slot):
    with tc.tile_pool(name=f"dc{li}_{slot}", bufs=2) as dpool:
        for b in range(BL):
            for cch in range(DCH):
                stripe = dpool.tile([128, SW], BF, tag="stripe")
                nc.scalar.copy(stripe[:, PAD + 1:PAD + 1 + L],
                               xT[:, cch, b * L:(b + 1) * L])
                nc.scalar.copy(
                    stripe[:, 0:PAD + 1],
                    xT[:, cch, b * L:b * L + 1].to_broadcast((128, PAD + 1)))
                nc.scalar.copy(
                    stripe[:, PAD + 1 + L:SW],
                    xT[:, cch, (b + 1) * L - 1:(b + 1) * L].to_broadcast((128, PAD)))
                init = dpool.tile([128, 1], F32, tag="init")
                nc.vector.reduce_sum(init[:], stripe[:, 0:K_MA], axis=AX.X)
                scano = dpool.tile([128, L], F32, tag="scano")
                nc.vector.tensor_tensor_scan(
                    scano[:], stripe[:, K_MA:SW], stripe[:, 0:L], init[:],
                    op0=ALU.add, op1=ALU.subtract)
                nc.vector.scalar_tensor_tensor(
                    out=xT[:, cch, b * L:(b + 1) * L], in0=scano[:],
                    scalar=-1.0 / K_MA, in1=xT[:, cch, b * L:(b + 1) * L],
                    op0=ALU.mult, op1=ALU.add,
                    accum_out=sl_x[:, cch, b, slot:slot + 1])


# ---------------------------------------------------------------------------
# entry point
# ---------------------------------------------------------------------------

_CACHE = {}


def kernel(x_enc, params, freq_index):
    consts = build_consts(params, freq_index)
    core_maps = stage_core_inputs(x_enc)
    if 'nc' not in _CACHE:
        _CACHE['nc'] = build_program()
    nc = _CACHE['nc']
    in_maps = [{**consts, **cm} for cm in core_maps]
    from concourse.bass_utils import run_bass_kernel_spmd
    res = run_bass_kernel_spmd(nc, in_maps, core_ids=list(range(NCORES)))
    out = np.concatenate([res.results[i]['out'] for i in range(NCORES)], axis=0)
    return out.astype(np.float32)


# revision 30
# speedup vs baseline: 1.4200x; 1.0506x over previous
"""DecompFEDformerEncoder Trainium2 kernel.

Data-parallel over batch (B=32 -> 4 per core x 8 cores), full model per core.
Residual stream x kept in [D(part-chunks), T] layout, bf16 master + fp32 accums.
Moving-average decomposition via tensor_tensor_scan sliding window.
Fourier block: DFT-as-matmul (64 low modes), per-(h,m) complex einsum as
weight-stationary [128x128] matmuls (complex folded into K), PE transposes,
iDFT-as-matmul. Trend stream never materialized: mean-over-L harvested from
fused accum_out on eviction ops via the telescoping identity
  enc = mean(x_final) + sum_j (mean z_j - mean x_j) + (MA0-mean @ tw + biases).
"""
import sys, os

for _p in ('/opt/trn_rl_repo', '/root/.axon_site/_ro/trn_rl_repo'):
    if os.path.isdir(_p) and _p not in sys.path:
        sys.path.insert(0, _p)

import numpy as np
import ml_dtypes

import concourse.bass as bass
import concourse.mybir as mybir
import concourse.tile as tile
from concourse import bacc

BF16 = ml_dtypes.bfloat16
F32 = mybir.dt.float32
BF = mybir.dt.bfloat16
ALU = mybir.AluOpType
ACTF = mybir.ActivationFunctionType
AX = mybir.AxisListType

# problem dims
B, L, FEAT = 32, 1024, 128
D, DFF, H, NLAYERS = 512, 2048, 8, 2
MODES, K_MA, CLS = 64, 25, 128
E = D // H               # 64
NCORES = 8
BL = B // NCORES         # 4 batches per core
T = BL * L               # 4096 tokens per core
PAD = (K_MA - 1) // 2    # 12
SW = L + K_MA            # stripe width 1049
DCH = D // 128           # 4 chunks of channels
NPAIR = H * MODES        # 512 einsum pairs


# ---------------------------------------------------------------------------
# host-side constant staging
# ---------------------------------------------------------------------------

def _posemb():
    pos = np.arange(L, dtype=np.float64)[:, None]
    div = np.exp(np.arange(0, D, 2, dtype=np.float64) * (-np.log(10000.0) / D))
    pe = np.zeros((L, D), np.float64)
    pe[:, 0::2] = np.sin(pos * div)
    pe[:, 1::2] = np.cos(pos * div)
    return pe.astype(np.float32)


def build_consts(params, freq_index):
    """Return dict of replicated (same on all cores) dram input arrays."""
    g = lambda a: np.asarray(a)
    fi = np.asarray(freq_index).astype(np.int64)   # [64] mode indices
    c = {}

    # DFT matrix: F2[l, m] = cos(2 pi f_m l / L); F2[l, 64+m] = -sin(...)
    ll = np.arange(L, dtype=np.float64)[:, None]
    ang = 2.0 * np.pi * fi[None, :] * ll / L       # [L, 64]
    F2 = np.concatenate([np.cos(ang), -np.sin(ang)], axis=1)  # [1024, 128]
    # swizzle to [128, 8, 128]: F2h[p, k, m2] = F2[k*128+p, m2]
    F2h = F2.reshape(8, 128, 128).transpose(1, 0, 2).astype(BF16).copy()

    # iDFT: out[l] = sum_m s_m (Re[m] cos - Im[m] sin), s_m = (2 - [f_m==0])/L
    s = (2.0 - (fi == 0).astype(np.float64)) / L
    lr = np.arange(L, dtype=np.float64)[None, :]
    angi = 2.0 * np.pi * fi[:, None] * lr / L      # [64, L]
    G2big = np.concatenate([s[:, None] * np.cos(angi),
                            -s[:, None] * np.sin(angi)], 0)  # [128=(r,m), L]
    c['F2'] = F2h

    layers = params['layers']
    qw_h, M2_h, c1_h, c2_h, W2_h, qbc_h = [], [], [], [], [], []
    obsum = np.zeros(D, np.float64)
    for lp in layers:
        qw = g(lp['qw']).astype(np.float64)        # [512, 512]
        qb = g(lp['qb']).astype(np.float64)        # [512]
        ow = g(lp['ow']).astype(np.float64)
        ob = g(lp['ob']).astype(np.float64)
        fwre = g(lp['fw_re']).astype(np.float64)   # [8, 64, 64, 64] (h,i,o,m)
        fwim = g(lp['fw_im']).astype(np.float64)
        c1 = g(lp['c1']).astype(np.float64)        # [512, 2048]
        c2 = g(lp['c2']).astype(np.float64)        # [2048, 512]
        obsum += ob

        qw_h.append(qw.reshape(4, 128, 512).transpose(1, 0, 2).reshape(128, 4 * 512))
        # fused iDFT + o-proj: M2[j][(r,m), n] = sum_d' G2big[(r,m), 512j+d'] ow[d', n]
        M2_h.append(np.stack([G2big[:, 512 * j:512 * (j + 1)] @ ow
                              for j in range(2)]))          # [2, 128, 512]
        c1_h.append(c1.reshape(4, 128, DFF).transpose(1, 0, 2).reshape(128, 4 * DFF))
        c2_h.append(c2.reshape(16, 128, 512).transpose(1, 0, 2).reshape(128, 16 * 512))

        # einsum stationary: for pair p=(h*64+m): Wt[kappa=(r_in*64+i), (r_out*64+o)]
        # [[re,  im],
        #  [-im, re]]
        Wt = np.zeros((NPAIR, 128, 128), np.float64)
        fre = fwre.transpose(0, 3, 1, 2)           # [h, m, i, o]
        fim = fwim.transpose(0, 3, 1, 2)
        fre = fre.reshape(NPAIR, 64, 64)
        fim = fim.reshape(NPAIR, 64, 64)
        Wt[:, 0:64, 0:64] = fre
        Wt[:, 0:64, 64:128] = fim
        Wt[:, 64:128, 0:64] = -fim
        Wt[:, 64:128, 64:128] = fre
        # swizzle [32 blocks, 128(kappa), 16, 128]: W2h[bl, p, j, o2]=Wt[bl*16+j, p, o2]
        W2_h.append(Wt.reshape(32, 16, 128, 128).transpose(0, 2, 1, 3)
                      .reshape(32, 128, 16 * 128))

        # q-bias correction on X: modes with f_m == 0 get += L * qb (real part)
        qbc = np.zeros((128, H), np.float64)
        qbc[0:64, :] = (L * qb).reshape(H, E).T    # rows (re, i), col h
        qbc_h.append(qbc)

    c['qw'] = np.stack(qw_h).astype(BF16)
    c['M2'] = np.stack(M2_h).astype(BF16)          # [2, 2, 128, 512]
    c['c1'] = np.stack(c1_h).astype(BF16)
    c['c2'] = np.stack(c2_h).astype(BF16)
    c['W2'] = np.stack(W2_h).astype(BF16)          # [2, 32, 128, 2048]
    c['qbc'] = np.stack(qbc_h).astype(BF16)        # [2, 128, 8]

    sw = g(params['seas_w']).astype(np.float64)    # [128, 512]
    sb = g(params['seas_b']).astype(np.float64)    # [512]
    tw = g(params['trend_w']).astype(np.float64)
    tb = g(params['trend_b']).astype(np.float64)
    cw = g(params['cls_w']).astype(np.float64)     # [512, 128]
    cb = g(params['cls_b']).astype(np.float64)     # [128]

    c['sw'] = sw.astype(BF16)                      # [128, 512] lhsT natural
    c['sbb'] = sb.reshape(4, 128).T.astype(np.float32).copy()   # [128, 4]
    c['tw'] = tw.astype(np.float32)                # [128, 512]
    c['tbias'] = (tb + obsum).reshape(4, 128).T.astype(np.float32).copy()
    c['clsw'] = cw.reshape(4, 128, CLS).transpose(1, 0, 2).reshape(128, 4 * CLS).astype(np.float32).copy()
    c['clsb'] = np.broadcast_to(cb.astype(np.float32), (BL, CLS)).copy()
    c['pos'] = _posemb().T.reshape(4, 128, L).transpose(1, 0, 2).reshape(128, 4 * L).astype(BF16).copy()
    c['identb'] = np.eye(128, dtype=BF16)
    return c


def stage_core_inputs(x_enc):
    """Per-core input staging: x in [feat, T] bf16 + fp32 MA stripes."""
    maps = []
    for ci in range(NCORES):
        xs = np.asarray(x_enc[ci * BL:(ci + 1) * BL]).astype(np.float32)  # [4,1024,128]
        xT = xs.reshape(T, FEAT).T.copy()                  # [128, 4096]
        stripes = np.zeros((FEAT, BL, SW), np.float32)
        for b in range(BL):
            xb = xs[b].T                                    # [128, 1024]
            stripes[:, b, PAD + 1:PAD + 1 + L] = xb
            stripes[:, b, 0:PAD + 1] = xb[:, 0:1]
            stripes[:, b, PAD + 1 + L:] = xb[:, -1:]
        maps.append({'x_encT': xT.astype(BF16),
                     'stripes0': stripes.reshape(FEAT, BL * SW)})
    return maps


# ---------------------------------------------------------------------------
# device program
# ---------------------------------------------------------------------------

def build_program():
    nc = bacc.Bacc("TRN2", target_bir_lowering=False, debug=False)

    dram = {}
    def din(name, shape, dt):
        dram[name] = nc.dram_tensor(name, list(shape), dt, kind="ExternalInput")
        return dram[name]

    x_encT_d = din('x_encT', [FEAT, T], BF)
    stripes0_d = din('stripes0', [FEAT, BL * SW], F32)
    F2_d = din('F2', [128, 8, 128], BF)
    qw_d = din('qw', [NLAYERS, 128, 4 * 512], BF)
    M2_d = din('M2', [NLAYERS, 2, 128, 512], BF)
    c1_d = din('c1', [NLAYERS, 128, 4 * DFF], BF)
    c2_d = din('c2', [NLAYERS, 128, 16 * 512], BF)
    W2_d = din('W2', [NLAYERS, 32, 128, 16 * 128], BF)
    qbc_d = din('qbc', [NLAYERS, 128, H], BF)
    sw_d = din('sw', [128, 512], BF)
    sbb_d = din('sbb', [128, 4], F32)
    tw_d = din('tw', [128, 512], F32)
    tbias_d = din('tbias', [128, 4], F32)
    clsw_d = din('clsw', [128, 4 * CLS], F32)
    clsb_d = din('clsb', [BL, CLS], F32)
    pos_d = din('pos', [128, 4 * L], BF)
    identb_d = din('identb', [128, 128], BF)
    out_d = nc.dram_tensor('out', [BL, CLS], F32, kind="ExternalOutput")

    with tile.TileContext(nc) as tc:
        # ------- persistent pools
        with tc.tile_pool(name="resid", bufs=1) as resid_pool, \
             tc.tile_pool(name="consts", bufs=1) as cpool, \
             tc.tile_pool(name="slots", bufs=1) as spool, \
             tc.tile_pool(name="wpool", bufs=1) as wpool:

            xT = resid_pool.tile([128, DCH, T], BF)        # residual stream
            F2s = cpool.tile([128, 8, 128], BF)
            nc.sync.dma_start(F2s[:], F2_d[:])
            sws = cpool.tile([128, 512], BF)
            nc.sync.dma_start(sws[:], sw_d[:])
            sbbs = cpool.tile([128, 4], F32)
            nc.sync.dma_start(sbbs[:], sbb_d[:])
            tws = cpool.tile([128, 512], F32)
            nc.sync.dma_start(tws[:], tw_d[:])
            tbs = cpool.tile([128, 4], F32)
            nc.sync.dma_start(tbs[:], tbias_d[:])
            clsws = cpool.tile([128, 4 * CLS], F32)
            nc.sync.dma_start(clsws[:], clsw_d[:])
            clsbs = cpool.tile([BL, CLS], F32)
            nc.sync.dma_start(clsbs[:], clsb_d[:])
            identb = cpool.tile([128, 128], BF)
            nc.sync.dma_start(identb[:], identb_d[:])

            # accumulation slots (all fp32, written exactly once each)
            # z-slots per (c,b): 0..3 attn(li*2+j), 4..7 ffn(li*2+half), 8 x0
            sl_z = spool.tile([128, DCH, BL, 9], F32)
            sl_x = spool.tile([128, DCH, BL, 4], F32)
            sl_seas0 = spool.tile([128, BL], F32)      # feat-space
            sl_xenc = spool.tile([128, BL], F32)       # feat-space

            # ---------------- stage 0: initial decomposition + embedding
            with tc.tile_pool(name="emb", bufs=1) as epool, \
                 tc.tile_pool(name="emb2", bufs=2) as epool2, \
                 tc.tile_pool(name="embps", bufs=2, space="PSUM") as eps:
                xe = epool.tile([128, T], BF)
                nc.sync.dma_start(xe[:], x_encT_d[:])
                str0 = epool.tile([128, BL, SW], F32)
                nc.sync.dma_start(str0[:], stripes0_d[:].rearrange(
                    "p (b s) -> p b s", b=BL))
                seas0 = epool.tile([128, T], BF)

                # sum_l x_enc per (feat, b)
                nc.vector.reduce_sum(sl_xenc[:], xe[:].rearrange(
                    "p (b l) -> p b l", b=BL), axis=AX.X)

                for b in range(BL):
                    init = epool2.tile([128, 1], F32, tag="init")
                    nc.vector.reduce_sum(init[:], str0[:, b, 0:K_MA], axis=AX.X)
                    scano = epool2.tile([128, L], F32, tag="scano")
                    nc.vector.tensor_tensor_scan(
                        scano[:], str0[:, b, K_MA:SW], str0[:, b, 0:L], init[:],
                        op0=ALU.add, op1=ALU.subtract)
                    # seas0 = x_enc - scano/25 ; accum = sum(seas0)
                    nc.vector.scalar_tensor_tensor(
                        out=seas0[:, b * L:(b + 1) * L], in0=scano[:],
                        scalar=-1.0 / K_MA, in1=xe[:, b * L:(b + 1) * L],
                        op0=ALU.mult, op1=ALU.add, accum_out=sl_seas0[:, b:b + 1])

                # embedding: xT[n-chunk] = sw.T @ seas0 + sb ; then += pos
                for n in range(DCH):
                    for s2 in range(8):
                        ps_t = eps.tile([128, 512], F32, tag="embps")
                        nc.tensor.matmul(ps_t[:], sws[:, n * 128:(n + 1) * 128],
                                         seas0[:, s2 * 512:(s2 + 1) * 512],
                                         start=True, stop=True)
                        nc.scalar.activation(xT[:, n, s2 * 512:(s2 + 1) * 512],
                                             ps_t[:], ACTF.Identity,
                                             bias=sbbs[:, n:n + 1])
                for n in range(DCH):
                    posc = epool2.tile([128, L], BF, tag="pos")
                    nc.sync.dma_start(posc[:], pos_d[:, n * L:(n + 1) * L])
                    for b in range(BL):
                        nc.vector.scalar_tensor_tensor(
                            out=xT[:, n, b * L:(b + 1) * L], in0=posc[:],
                            scalar=1.0, in1=xT[:, n, b * L:(b + 1) * L],
                            op0=ALU.mult, op1=ALU.add,
                            accum_out=sl_z[:, n, b, 8:9])   # slot: sum(x0)

            # ---------------- layers
            for li in range(NLAYERS):
                qws = wpool.tile([128, 4, 512], BF, tag="qw")
                nc.sync.dma_start(qws[:], qw_d[li].rearrange("p (k n) -> p k n", k=4))
                m2s = wpool.tile([128, 2, 512], BF, tag="m2")
                nc.sync.dma_start(m2s[:], M2_d[li])
                c1s = wpool.tile([128, 4, DFF], BF, tag="c1")
                nc.sync.dma_start(c1s[:], c1_d[li].rearrange("p (k n) -> p k n", k=4))
                c2s = wpool.tile([128, 16, 512], BF, tag="c2")
                nc.sync.dma_start(c2s[:], c2_d[li].rearrange("p (k n) -> p k n", k=16))
                qbcs = wpool.tile([128, H], BF, tag="qbc")
                nc.sync.dma_start(qbcs[:], qbc_d[li])

                _layer(nc, tc, li, xT, F2s, identb, qws, m2s, c1s, c2s,
                       qbcs, W2_d, sl_z, sl_x)

            # ---------------- final: combine means, classifier
            with tc.tile_pool(name="fin", bufs=1) as fpool, \
                 tc.tile_pool(name="finps", bufs=2, space="PSUM") as fps:
                zsum = fpool.tile([128, DCH, BL], F32)
                nc.vector.reduce_sum(zsum[:], sl_z[:, :, :, 0:8], axis=AX.X)
                xsum = fpool.tile([128, DCH, BL], F32)
                nc.vector.reduce_sum(xsum[:], sl_x[:, :, :, 0:3], axis=AX.X)
                acc = fpool.tile([128, DCH, BL], F32)
                nc.vector.tensor_tensor(acc[:], zsum[:], xsum[:], op=ALU.subtract)

                # meanMA0 = (sum x_enc - sum seas0)/L  [feat, b]
                mm0 = fpool.tile([128, BL], F32)
                nc.vector.tensor_tensor(mm0[:], sl_xenc[:], sl_seas0[:], op=ALU.subtract)
                nc.vector.tensor_scalar_mul(mm0[:], mm0[:], 1.0 / L)

                enc = fpool.tile([128, DCH, BL], F32)
                for cch in range(DCH):
                    ps_tw = fps.tile([128, BL], F32, tag="twps")
                    nc.tensor.matmul(ps_tw[:], tws[:, cch * 128:(cch + 1) * 128],
                                     mm0[:], start=True, stop=True)
                    # enc = acc/L + tw-term
                    nc.vector.scalar_tensor_tensor(
                        out=enc[:, cch, :], in0=acc[:, cch, :], scalar=1.0 / L,
                        in1=ps_tw[:], op0=ALU.mult, op1=ALU.add)
                    nc.vector.tensor_scalar(
                        enc[:, cch, :], enc[:, cch, :], tbs[:, cch:cch + 1], None,
                        op0=ALU.add)

                ps_cls = fps.tile([BL, CLS], F32, tag="clsps")
                for cch in range(DCH):
                    nc.tensor.matmul(ps_cls[:], enc[:, cch, :],
                                     clsws[:, cch * CLS:(cch + 1) * CLS],
                                     start=(cch == 0), stop=(cch == DCH - 1))
                outs = fpool.tile([BL, CLS], F32)
                nc.vector.tensor_tensor(outs[:], ps_cls[:], clsbs[:], op=ALU.add)
                nc.sync.dma_start(out_d[:], outs[:])

    nc.compile()
    return nc


def _layer(nc, tc, li, xT, F2s, identb, qws, m2s, c1s, c2s, qbcs,
           W2_d, sl_z, sl_x):
    """Emit one encoder layer."""
    # ============ fourier attention ============
    with tc.tile_pool(name=f"att{li}", bufs=1) as apool, \
         tc.tile_pool(name=f"att2{li}", bufs=2) as apool2:

        Xstk = apool.tile([128, NPAIR * BL], BF)          # [(r,i), (h,m,b)]
        X4 = Xstk[:].rearrange("p (h m b) -> p h m b", h=H, m=MODES, b=BL)

        # ---- q projection + DFT, per batch
        aps_cm = tc.tile_pool(name=f"attps{li}", bufs=2, space="PSUM")
        aps = aps_cm.__enter__()
        for b in range(BL):
            qsb = apool2.tile([128, 8, 512], BF, tag="qsb")
            for lc in range(8):
                tg = (b * 8 + lc) * 128
                ps_q = aps.tile([128, 512], F32, tag="qps")
                for k in range(DCH):
                    nc.tensor.matmul(ps_q[:], xT[:, k, tg:tg + 128], qws[:, k, :],
                                     start=(k == 0), stop=(k == DCH - 1))
                nc.scalar.activation(qsb[:, lc, :], ps_q[:], ACTF.Copy, bias=0.0)
            for cch in range(DCH):
                ps_x = aps.tile([128, 128], F32, tag="xps")
                for lc in range(8):
                    nc.tensor.matmul(ps_x[:], qsb[:, lc, cch * 128:(cch + 1) * 128],
                                     F2s[:, lc, :], start=(lc == 0), stop=(lc == 7))
                h0 = 2 * cch
                # re rows (0:64) <- psum cols 0:64 ; im rows (64:128) <- cols 64:128
                nc.vector.tensor_copy(X4[0:64, h0, :, b], ps_x[0:64, 0:64])
                nc.vector.tensor_copy(X4[0:64, h0 + 1, :, b], ps_x[64:128, 0:64])
                nc.vector.tensor_copy(X4[64:128, h0, :, b], ps_x[0:64, 64:128])
                nc.vector.tensor_copy(X4[64:128, h0 + 1, :, b], ps_x[64:128, 64:128])

        # q-bias correction on zero-frequency modes (mode list baked in host F2;
        # correction column m=0 matches freq_index arange convention)
        nc.vector.tensor_tensor(X4[:, :, 0, :], X4[:, :, 0, :],
                                qbcs[:, :, None].to_broadcast((128, H, BL)),
                                op=ALU.add)
        aps_cm.__exit__(None, None, None)

        # ---- einsum: 512 weight-stationary matmuls
        aps_cm = tc.tile_pool(name=f"attps{li}e", bufs=2, space="PSUM")
        aps = aps_cm.__enter__()
        apt_cm = tc.tile_pool(name=f"attps{li}t", bufs=2, space="PSUM")
        apt = apt_cm.__enter__()
        OutSel = apool.tile([128, NPAIR * BL], BF)        # [(r,o), (pair,b)]
        OutT = apool.tile([128, BL * H * E], BF)          # [(r,m), (b,h,o)]
        OT4 = OutT[:].rearrange("p (b h o) -> p b h o", b=BL, h=H)
        OS4 = OutSel[:].rearrange("p (pr b) -> p pr b", b=BL)
        for g4 in range(4):                                # psum bank groups
            ps_e = aps.tile([128, 512], F32, tag="eps")
            for bl in range(g4 * 8, (g4 + 1) * 8):         # 8 weight blocks of 16
                wsb = apool2.tile([128, 16 * 128], BF, tag="wsb")
                nc.sync.dma_start(wsb[:], W2_d[li, bl])
                for j in range(16):
                    p = bl * 16 + j
                    nc.tensor.matmul(ps_e[:, (p % 128) * 4:(p % 128) * 4 + 4],
                                     wsb[:, j * 128:(j + 1) * 128],
                                     Xstk[:, p * 4:(p + 1) * 4],
                                     start=True, stop=True)
            nc.vector.tensor_copy(OutSel[:, g4 * 512:(g4 + 1) * 512], ps_e[:])
            # transpose the two heads this group just produced into
            # OutT2[(r,m), (b, h, o)] (r via partition-offset evictions)
            for hh in (2 * g4, 2 * g4 + 1):
                for b in range(BL):
                    for r in range(2):
                        ps_t = apt.tile([64, 64], BF, tag="tps")
                        nc.tensor.transpose(
                            ps_t[:], OS4[r * 64:(r + 1) * 64,
                                         hh * 64:(hh + 1) * 64, b],
                            identb[r * 64:(r + 1) * 64, r * 64:(r + 1) * 64])
                        nc.vector.tensor_copy(OT4[r * 64:(r + 1) * 64, b, hh, :],
                                              ps_t[:])
        apt_cm.__exit__(None, None, None)
        aps_cm.__exit__(None, None, None)

        # ---- per-batch pipelined back-half:
        #      fused o-proj(b) -> decomp1(b) -> FFN(2b, 2b+1) -> decomp2(b)
        aps_cm = tc.tile_pool(name=f"attps{li}v", bufs=2, space="PSUM")
        aps = aps_cm.__enter__()
        fpool_cm = tc.tile_pool(name=f"ffn{li}", bufs=2)
        fpool = fpool_cm.__enter__()
        dpool_cm = tc.tile_pool(name=f"dc{li}", bufs=3)
        dpool = dpool_cm.__enter__()
        fps_cm = tc.tile_pool(name=f"ffnps{li}", bufs=2, space="PSUM")
        fps = fps_cm.__enter__()
        fpsg_cm = tc.tile_pool(name=f"ffnpsg{li}", bufs=4, space="PSUM")
        fpsg = fpsg_cm.__enter__()
        for b in range(BL):
            for j in range(2):
                for n in range(DCH):
                    ps_o = aps.tile([128, 512], F32, tag="ops")
                    nc.tensor.matmul(ps_o[:], m2s[:, j, n * 128:(n + 1) * 128],
                                     OutT[:, b * 512:(b + 1) * 512],
                                     start=True, stop=True)
                    xv = xT[:, n, b * L:(b + 1) * L].rearrange(
                        "p (h o two) -> p h o two", h=H, o=E, two=2)[:, :, :, j]
                    nc.vector.scalar_tensor_tensor(
                        out=xv, in0=ps_o[:].rearrange("p (h o) -> p h o", h=H),
                        scalar=1.0, in1=xv, op0=ALU.mult, op1=ALU.add,
                        accum_out=sl_z[:, n, b, li * 2 + j:li * 2 + j + 1])
            _decomp_b(nc, dpool, xT, sl_x, b, slot=li * 2 + 0)
            for s2 in (2 * b, 2 * b + 1):
                ps_g = [fpsg.tile([128, 512], F32, tag="gps", name=f"psg{n}")
                        for n in range(DCH)]
                for f in range(4):
                    y1 = fpool.tile([128, 4, 512], BF, tag="y1")
                    for mm in range(4):
                        ps_f = fps.tile([128, 512], F32, tag="fps")
                        for k in range(DCH):
                            nc.tensor.matmul(
                                ps_f[:], c1s[:, k, f * 512 + mm * 128:f * 512 + (mm + 1) * 128],
                                xT[:, k, s2 * 512:(s2 + 1) * 512],
                                start=(k == 0), stop=(k == DCH - 1))
                        nc.scalar.activation(y1[:, mm, :], ps_f[:], ACTF.Gelu, bias=0.0)
                    for n in range(DCH):
                        for mm in range(4):
                            nc.tensor.matmul(ps_g[n][:],
                                             c2s[:, f * 4 + mm, n * 128:(n + 1) * 128],
                                             y1[:, mm, :],
                                             start=(f == 0 and mm == 0),
                                             stop=(f == 3 and mm == 3))
                half = s2 % 2
                zslot = 4 + li * 2 + half
                for n in range(DCH):
                    xsl = xT[:, n, s2 * 512:(s2 + 1) * 512]
                    nc.vector.scalar_tensor_tensor(
                        out=xsl, in0=ps_g[n][:], scalar=1.0, in1=xsl,
                        op0=ALU.mult, op1=ALU.add,
                        accum_out=sl_z[:, n, b, zslot:zslot + 1])
            # decomp2 (skipped for the last layer: the final seasonal x4 only
            # enters the output via mean(x4) + mean(MA(z4)) = mean(z4))
            if li < NLAYERS - 1:
                _decomp_b(nc, dpool, xT, sl_x, b, slot=li * 2 + 1)
        fpsg_cm.__exit__(None, None, None)
        fps_cm.__exit__(None, None, None)
        dpool_cm.__exit__(None, None, None)
        fpool_cm.__exit__(None, None, None)
        aps_cm.__exit__(None, None, None)


def _decomp_b(nc, dpool, xT, sl_x, b, slot):
        if True:
            for cch in range(DCH):
                stripe = dpool.tile([128, SW], BF, tag="stripe")
                nc.scalar.copy(stripe[:, PAD + 1:PAD + 1 + L],
                               xT[:, cch, b * L:(b + 1) * L])
                nc.scalar.copy(
                    stripe[:, 0:PAD + 1],
                    xT[:, cch, b * L:b * L + 1].to_broadcast((128, PAD + 1)))
                nc.scalar.copy(
                    stripe[:, PAD + 1 + L:SW],
                    xT[:, cch, (b + 1) * L - 1:(b + 1) * L].to_broadcast((128, PAD)))
                init = dpool.tile([128, 1], F32, tag="init")
                nc.vector.reduce_sum(init[:], stripe[:, 0:K_MA], axis=AX.X)
                scano = dpool.tile([128, L], F32, tag="scano")
                nc.vector.tensor_tensor_scan(
                    scano[:], stripe[:, K_MA:SW], stripe[:, 0:L], init[:],
                    op0=ALU.add, op1=ALU.subtract)
                nc.vector.scalar_tensor_tensor(
                    out=xT[:, cch, b * L:(b + 1) * L], in0=scano[:],
                    scalar=-1.0 / K_MA, in1=xT[:, cch, b * L:(b + 1) * L],
                    op0=ALU.mult, op1=ALU.add,
                    accum_out=sl_x[:, cch, b, slot:slot + 1])


# ---------------------------------------------------------------------------
# entry point
# ---------------------------------------------------------------------------

_CACHE = {}


def kernel(x_enc, params, freq_index):
    consts = build_consts(params, freq_index)
    core_maps = stage_core_inputs(x_enc)
    if 'nc' not in _CACHE:
        _CACHE['nc'] = build_program()
    nc = _CACHE['nc']
    in_maps = [{**consts, **cm} for cm in core_maps]
    from concourse.bass_utils import run_bass_kernel_spmd
    res = run_bass_kernel_spmd(nc, in_maps, core_ids=list(range(NCORES)))
    out = np.concatenate([res.results[i]['out'] for i in range(NCORES)], axis=0)
    return out.astype(np.float32)


# revision 32
# speedup vs baseline: 1.4649x; 1.0316x over previous
"""DecompFEDformerEncoder Trainium2 kernel.

Data-parallel over batch (B=32 -> 4 per core x 8 cores), full model per core.
Residual stream x kept in [D(part-chunks), T] layout, bf16 master + fp32 accums.
Moving-average decomposition via tensor_tensor_scan sliding window.
Fourier block: DFT-as-matmul (64 low modes), per-(h,m) complex einsum as
weight-stationary [128x128] matmuls (complex folded into K), PE transposes,
iDFT-as-matmul. Trend stream never materialized: mean-over-L harvested from
fused accum_out on eviction ops via the telescoping identity
  enc = mean(x_final) + sum_j (mean z_j - mean x_j) + (MA0-mean @ tw + biases).
"""
import sys, os

for _p in ('/opt/trn_rl_repo', '/root/.axon_site/_ro/trn_rl_repo'):
    if os.path.isdir(_p) and _p not in sys.path:
        sys.path.insert(0, _p)

import numpy as np
import ml_dtypes

import concourse.bass as bass
import concourse.mybir as mybir
import concourse.tile as tile
from concourse import bacc

BF16 = ml_dtypes.bfloat16
F32 = mybir.dt.float32
BF = mybir.dt.bfloat16
ALU = mybir.AluOpType
ACTF = mybir.ActivationFunctionType
AX = mybir.AxisListType

# problem dims
B, L, FEAT = 32, 1024, 128
D, DFF, H, NLAYERS = 512, 2048, 8, 2
MODES, K_MA, CLS = 64, 25, 128
E = D // H               # 64
NCORES = 8
BL = B // NCORES         # 4 batches per core
T = BL * L               # 4096 tokens per core
PAD = (K_MA - 1) // 2    # 12
SW = L + K_MA            # stripe width 1049
DCH = D // 128           # 4 chunks of channels
NPAIR = H * MODES        # 512 einsum pairs


# ---------------------------------------------------------------------------
# host-side constant staging
# ---------------------------------------------------------------------------

def _posemb():
    pos = np.arange(L, dtype=np.float64)[:, None]
    div = np.exp(np.arange(0, D, 2, dtype=np.float64) * (-np.log(10000.0) / D))
    pe = np.zeros((L, D), np.float64)
    pe[:, 0::2] = np.sin(pos * div)
    pe[:, 1::2] = np.cos(pos * div)
    return pe.astype(np.float32)


def build_consts(params, freq_index):
    """Return dict of replicated (same on all cores) dram input arrays."""
    g = lambda a: np.asarray(a)
    fi = np.asarray(freq_index).astype(np.int64)   # [64] mode indices
    c = {}

    # DFT matrix: F2[l, m] = cos(2 pi f_m l / L); F2[l, 64+m] = -sin(...)
    ll = np.arange(L, dtype=np.float64)[:, None]
    ang = 2.0 * np.pi * fi[None, :] * ll / L       # [L, 64]
    F2 = np.concatenate([np.cos(ang), -np.sin(ang)], axis=1)  # [1024, 128]
    # swizzle to [128, 8, 128]: F2h[p, k, m2] = F2[k*128+p, m2]
    F2h = F2.reshape(8, 128, 128).transpose(1, 0, 2).astype(BF16).copy()

    # iDFT: out[l] = sum_m s_m (Re[m] cos - Im[m] sin), s_m = (2 - [f_m==0])/L
    s = (2.0 - (fi == 0).astype(np.float64)) / L
    lr = np.arange(L, dtype=np.float64)[None, :]
    angi = 2.0 * np.pi * fi[:, None] * lr / L      # [64, L]
    G2big = np.concatenate([s[:, None] * np.cos(angi),
                            -s[:, None] * np.sin(angi)], 0)  # [128=(r,m), L]
    c['F2'] = F2h

    layers = params['layers']
    qw_h, M2_h, c1_h, c2_h, W2_h, qbc_h = [], [], [], [], [], []
    obsum = np.zeros(D, np.float64)
    for lp in layers:
        qw = g(lp['qw']).astype(np.float64)        # [512, 512]
        qb = g(lp['qb']).astype(np.float64)        # [512]
        ow = g(lp['ow']).astype(np.float64)
        ob = g(lp['ob']).astype(np.float64)
        fwre = g(lp['fw_re']).astype(np.float64)   # [8, 64, 64, 64] (h,i,o,m)
        fwim = g(lp['fw_im']).astype(np.float64)
        c1 = g(lp['c1']).astype(np.float64)        # [512, 2048]
        c2 = g(lp['c2']).astype(np.float64)        # [2048, 512]
        obsum += ob

        qw_h.append(qw.reshape(4, 128, 512).transpose(1, 0, 2).reshape(128, 4 * 512))
        # fused iDFT + o-proj: M2[j][(r,m), n] = sum_d' G2big[(r,m), 512j+d'] ow[d', n]
        M2_h.append(np.stack([G2big[:, 512 * j:512 * (j + 1)] @ ow
                              for j in range(2)]))          # [2, 128, 512]
        c1_h.append(c1.reshape(4, 128, DFF).transpose(1, 0, 2).reshape(128, 4 * DFF))
        c2_h.append(c2.reshape(16, 128, 512).transpose(1, 0, 2).reshape(128, 16 * 512))

        # einsum stationary: for pair p=(h*64+m): Wt[kappa=(r_in*64+i), (r_out*64+o)]
        # [[re,  im],
        #  [-im, re]]
        Wt = np.zeros((NPAIR, 128, 128), np.float64)
        fre = fwre.transpose(0, 3, 1, 2)           # [h, m, i, o]
        fim = fwim.transpose(0, 3, 1, 2)
        fre = fre.reshape(NPAIR, 64, 64)
        fim = fim.reshape(NPAIR, 64, 64)
        Wt[:, 0:64, 0:64] = fre
        Wt[:, 0:64, 64:128] = fim
        Wt[:, 64:128, 0:64] = -fim
        Wt[:, 64:128, 64:128] = fre
        # swizzle [32 blocks, 128(kappa), 16, 128]: W2h[bl, p, j, o2]=Wt[bl*16+j, p, o2]
        W2_h.append(Wt.reshape(32, 16, 128, 128).transpose(0, 2, 1, 3)
                      .reshape(32, 128, 16 * 128))

        # q-bias correction on X: modes with f_m == 0 get += L * qb (real part)
        qbc = np.zeros((128, H), np.float64)
        qbc[0:64, :] = (L * qb).reshape(H, E).T    # rows (re, i), col h
        qbc_h.append(qbc)

    c['qw'] = np.stack(qw_h).astype(BF16)
    c['M2'] = np.stack(M2_h).astype(BF16)          # [2, 2, 128, 512]
    c['c1'] = np.stack(c1_h).astype(BF16)
    c['c2'] = np.stack(c2_h).astype(BF16)
    c['W2'] = np.stack(W2_h).astype(BF16)          # [2, 32, 128, 2048]
    c['qbc'] = np.stack(qbc_h).astype(BF16)        # [2, 128, 8]

    sw = g(params['seas_w']).astype(np.float64)    # [128, 512]
    sb = g(params['seas_b']).astype(np.float64)    # [512]
    tw = g(params['trend_w']).astype(np.float64)
    tb = g(params['trend_b']).astype(np.float64)
    cw = g(params['cls_w']).astype(np.float64)     # [512, 128]
    cb = g(params['cls_b']).astype(np.float64)     # [128]

    c['sw'] = sw.astype(BF16)                      # [128, 512] lhsT natural
    c['sbb'] = sb.reshape(4, 128).T.astype(np.float32).copy()   # [128, 4]
    c['tw'] = tw.astype(np.float32)                # [128, 512]
    c['tbias'] = (tb + obsum).reshape(4, 128).T.astype(np.float32).copy()
    c['clsw'] = cw.reshape(4, 128, CLS).transpose(1, 0, 2).reshape(128, 4 * CLS).astype(np.float32).copy()
    c['clsb'] = np.broadcast_to(cb.astype(np.float32), (BL, CLS)).copy()
    c['pos'] = _posemb().T.reshape(4, 128, L).transpose(1, 0, 2).reshape(128, 4 * L).astype(BF16).copy()
    c['identb'] = np.eye(128, dtype=BF16)
    return c


def stage_core_inputs(x_enc):
    """Per-core input staging: x in [feat, T] bf16 + fp32 MA stripes."""
    maps = []
    for ci in range(NCORES):
        xs = np.asarray(x_enc[ci * BL:(ci + 1) * BL]).astype(np.float32)  # [4,1024,128]
        xT = xs.reshape(T, FEAT).T.copy()                  # [128, 4096]
        stripes = np.zeros((FEAT, BL, SW), np.float32)
        for b in range(BL):
            xb = xs[b].T                                    # [128, 1024]
            stripes[:, b, PAD + 1:PAD + 1 + L] = xb
            stripes[:, b, 0:PAD + 1] = xb[:, 0:1]
            stripes[:, b, PAD + 1 + L:] = xb[:, -1:]
        maps.append({'x_encT': xT.astype(BF16),
                     'stripes0': stripes.reshape(FEAT, BL * SW)})
    return maps


# ---------------------------------------------------------------------------
# device program
# ---------------------------------------------------------------------------

def build_program():
    nc = bacc.Bacc("TRN2", target_bir_lowering=False, debug=False)

    dram = {}
    def din(name, shape, dt):
        dram[name] = nc.dram_tensor(name, list(shape), dt, kind="ExternalInput")
        return dram[name]

    x_encT_d = din('x_encT', [FEAT, T], BF)
    stripes0_d = din('stripes0', [FEAT, BL * SW], F32)
    F2_d = din('F2', [128, 8, 128], BF)
    qw_d = din('qw', [NLAYERS, 128, 4 * 512], BF)
    M2_d = din('M2', [NLAYERS, 2, 128, 512], BF)
    c1_d = din('c1', [NLAYERS, 128, 4 * DFF], BF)
    c2_d = din('c2', [NLAYERS, 128, 16 * 512], BF)
    W2_d = din('W2', [NLAYERS, 32, 128, 16 * 128], BF)
    qbc_d = din('qbc', [NLAYERS, 128, H], BF)
    sw_d = din('sw', [128, 512], BF)
    sbb_d = din('sbb', [128, 4], F32)
    tw_d = din('tw', [128, 512], F32)
    tbias_d = din('tbias', [128, 4], F32)
    clsw_d = din('clsw', [128, 4 * CLS], F32)
    clsb_d = din('clsb', [BL, CLS], F32)
    pos_d = din('pos', [128, 4 * L], BF)
    identb_d = din('identb', [128, 128], BF)
    out_d = nc.dram_tensor('out', [BL, CLS], F32, kind="ExternalOutput")

    with tile.TileContext(nc) as tc:
        # ------- persistent pools
        with tc.tile_pool(name="resid", bufs=1) as resid_pool, \
             tc.tile_pool(name="consts", bufs=1) as cpool, \
             tc.tile_pool(name="slots", bufs=1) as spool, \
             tc.tile_pool(name="wpool", bufs=1) as wpool:

            xT = resid_pool.tile([128, DCH, T], BF)        # residual stream
            F2s = cpool.tile([128, 8, 128], BF)
            nc.sync.dma_start(F2s[:], F2_d[:])
            sws = cpool.tile([128, 512], BF)
            nc.sync.dma_start(sws[:], sw_d[:])
            sbbs = cpool.tile([128, 4], F32)
            nc.sync.dma_start(sbbs[:], sbb_d[:])
            tws = cpool.tile([128, 512], F32)
            nc.sync.dma_start(tws[:], tw_d[:])
            tbs = cpool.tile([128, 4], F32)
            nc.sync.dma_start(tbs[:], tbias_d[:])
            clsws = cpool.tile([128, 4 * CLS], F32)
            nc.sync.dma_start(clsws[:], clsw_d[:])
            clsbs = cpool.tile([BL, CLS], F32)
            nc.sync.dma_start(clsbs[:], clsb_d[:])
            identb = cpool.tile([128, 128], BF)
            nc.sync.dma_start(identb[:], identb_d[:])

            # accumulation slots (all fp32, written exactly once each)
            # z-slots per (c,b): 0..3 attn(li*2+j), 4..7 ffn(li*2+half), 8 x0
            sl_z = spool.tile([128, DCH, BL, 9], F32)
            sl_x = spool.tile([128, DCH, BL, 4], F32)
            sl_seas0 = spool.tile([128, BL], F32)      # feat-space
            sl_xenc = spool.tile([128, BL], F32)       # feat-space

            # ---------------- stage 0: initial decomposition + embedding
            with tc.tile_pool(name="emb", bufs=1) as epool, \
                 tc.tile_pool(name="emb2", bufs=2) as epool2, \
                 tc.tile_pool(name="embps", bufs=2, space="PSUM") as eps:
                xe = epool.tile([128, T], BF)
                nc.sync.dma_start(xe[:], x_encT_d[:])
                str0 = epool.tile([128, BL, SW], F32)
                nc.sync.dma_start(str0[:], stripes0_d[:].rearrange(
                    "p (b s) -> p b s", b=BL))
                seas0 = epool.tile([128, T], BF)

                # sum_l x_enc per (feat, b)
                nc.vector.reduce_sum(sl_xenc[:], xe[:].rearrange(
                    "p (b l) -> p b l", b=BL), axis=AX.X)

                for b in range(BL):
                    init = epool2.tile([128, 1], F32, tag="init")
                    nc.vector.reduce_sum(init[:], str0[:, b, 0:K_MA], axis=AX.X)
                    scano = epool2.tile([128, L], F32, tag="scano")
                    nc.vector.tensor_tensor_scan(
                        scano[:], str0[:, b, K_MA:SW], str0[:, b, 0:L], init[:],
                        op0=ALU.add, op1=ALU.subtract)
                    # seas0 = x_enc - scano/25 ; accum = sum(seas0)
                    nc.vector.scalar_tensor_tensor(
                        out=seas0[:, b * L:(b + 1) * L], in0=scano[:],
                        scalar=-1.0 / K_MA, in1=xe[:, b * L:(b + 1) * L],
                        op0=ALU.mult, op1=ALU.add, accum_out=sl_seas0[:, b:b + 1])

                # embedding: xT[n-chunk] = sw.T @ seas0 + sb ; then += pos
                for n in range(DCH):
                    for s2 in range(8):
                        ps_t = eps.tile([128, 512], F32, tag="embps")
                        nc.tensor.matmul(ps_t[:], sws[:, n * 128:(n + 1) * 128],
                                         seas0[:, s2 * 512:(s2 + 1) * 512],
                                         start=True, stop=True)
                        nc.scalar.activation(xT[:, n, s2 * 512:(s2 + 1) * 512],
                                             ps_t[:], ACTF.Identity,
                                             bias=sbbs[:, n:n + 1])
                for n in range(DCH):
                    posc = epool2.tile([128, L], BF, tag="pos")
                    nc.sync.dma_start(posc[:], pos_d[:, n * L:(n + 1) * L])
                    for b in range(BL):
                        nc.vector.scalar_tensor_tensor(
                            out=xT[:, n, b * L:(b + 1) * L], in0=posc[:],
                            scalar=1.0, in1=xT[:, n, b * L:(b + 1) * L],
                            op0=ALU.mult, op1=ALU.add,
                            accum_out=sl_z[:, n, b, 8:9])   # slot: sum(x0)

            # ---------------- layers
            for li in range(NLAYERS):
                qws = wpool.tile([128, 4, 512], BF, tag="qw")
                nc.sync.dma_start(qws[:], qw_d[li].rearrange("p (k n) -> p k n", k=4))
                m2s = wpool.tile([128, 2, 512], BF, tag="m2")
                nc.sync.dma_start(m2s[:], M2_d[li])
                c1s = wpool.tile([128, 4, DFF], BF, tag="c1")
                nc.sync.dma_start(c1s[:], c1_d[li].rearrange("p (k n) -> p k n", k=4))
                c2s = wpool.tile([128, 16, 512], BF, tag="c2")
                nc.sync.dma_start(c2s[:], c2_d[li].rearrange("p (k n) -> p k n", k=16))
                qbcs = wpool.tile([128, H], BF, tag="qbc")
                nc.sync.dma_start(qbcs[:], qbc_d[li])

                _layer(nc, tc, li, xT, F2s, identb, qws, m2s, c1s, c2s,
                       qbcs, W2_d, sl_z, sl_x)

            # ---------------- final: combine means, classifier
            with tc.tile_pool(name="fin", bufs=1) as fpool, \
                 tc.tile_pool(name="finps", bufs=2, space="PSUM") as fps:
                zsum = fpool.tile([128, DCH, BL], F32)
                nc.vector.reduce_sum(zsum[:], sl_z[:, :, :, 0:8], axis=AX.X)
                xsum = fpool.tile([128, DCH, BL], F32)
                nc.vector.reduce_sum(xsum[:], sl_x[:, :, :, 0:3], axis=AX.X)
                acc = fpool.tile([128, DCH, BL], F32)
                nc.vector.tensor_tensor(acc[:], zsum[:], xsum[:], op=ALU.subtract)

                # meanMA0 = (sum x_enc - sum seas0)/L  [feat, b]
                mm0 = fpool.tile([128, BL], F32)
                nc.vector.tensor_tensor(mm0[:], sl_xenc[:], sl_seas0[:], op=ALU.subtract)
                nc.vector.tensor_scalar_mul(mm0[:], mm0[:], 1.0 / L)

                enc = fpool.tile([128, DCH, BL], F32)
                for cch in range(DCH):
                    ps_tw = fps.tile([128, BL], F32, tag="twps")
                    nc.tensor.matmul(ps_tw[:], tws[:, cch * 128:(cch + 1) * 128],
                                     mm0[:], start=True, stop=True)
                    # enc = acc/L + tw-term
                    nc.vector.scalar_tensor_tensor(
                        out=enc[:, cch, :], in0=acc[:, cch, :], scalar=1.0 / L,
                        in1=ps_tw[:], op0=ALU.mult, op1=ALU.add)
                    nc.vector.tensor_scalar(
                        enc[:, cch, :], enc[:, cch, :], tbs[:, cch:cch + 1], None,
                        op0=ALU.add)

                ps_cls = fps.tile([BL, CLS], F32, tag="clsps")
                for cch in range(DCH):
                    nc.tensor.matmul(ps_cls[:], enc[:, cch, :],
                                     clsws[:, cch * CLS:(cch + 1) * CLS],
                                     start=(cch == 0), stop=(cch == DCH - 1))
                outs = fpool.tile([BL, CLS], F32)
                nc.vector.tensor_tensor(outs[:], ps_cls[:], clsbs[:], op=ALU.add)
                nc.sync.dma_start(out_d[:], outs[:])

    nc.compile()
    return nc


def _layer(nc, tc, li, xT, F2s, identb, qws, m2s, c1s, c2s, qbcs,
           W2_d, sl_z, sl_x):
    """Emit one encoder layer."""
    # ============ fourier attention ============
    with tc.tile_pool(name=f"att{li}", bufs=1) as apool, \
         tc.tile_pool(name=f"att2{li}", bufs=2) as apool2:

        Xstk = apool.tile([128, NPAIR * BL], BF)          # [(r,i), (h,m,b)]
        X4 = Xstk[:].rearrange("p (h m b) -> p h m b", h=H, m=MODES, b=BL)

        # ---- q projection + DFT, per batch
        aps_cm = tc.tile_pool(name=f"attps{li}", bufs=2, space="PSUM")
        aps = aps_cm.__enter__()
        for b in range(BL):
            qsb = apool2.tile([128, 8, 512], BF, tag="qsb")
            for lc in range(8):
                tg = (b * 8 + lc) * 128
                ps_q = aps.tile([128, 512], F32, tag="qps")
                for k in range(DCH):
                    nc.tensor.matmul(ps_q[:], xT[:, k, tg:tg + 128], qws[:, k, :],
                                     start=(k == 0), stop=(k == DCH - 1))
                nc.scalar.activation(qsb[:, lc, :], ps_q[:], ACTF.Copy, bias=0.0)
            for cch in range(DCH):
                ps_x = aps.tile([128, 128], F32, tag="xps")
                for lc in range(8):
                    nc.tensor.matmul(ps_x[:], qsb[:, lc, cch * 128:(cch + 1) * 128],
                                     F2s[:, lc, :], start=(lc == 0), stop=(lc == 7))
                h0 = 2 * cch
                # re rows (0:64) <- psum cols 0:64 ; im rows (64:128) <- cols 64:128
                nc.vector.tensor_copy(X4[0:64, h0, :, b], ps_x[0:64, 0:64])
                nc.vector.tensor_copy(X4[0:64, h0 + 1, :, b], ps_x[64:128, 0:64])
                nc.vector.tensor_copy(X4[64:128, h0, :, b], ps_x[0:64, 64:128])
                nc.vector.tensor_copy(X4[64:128, h0 + 1, :, b], ps_x[64:128, 64:128])

        # q-bias correction on zero-frequency modes (mode list baked in host F2;
        # correction column m=0 matches freq_index arange convention)
        nc.vector.tensor_tensor(X4[:, :, 0, :], X4[:, :, 0, :],
                                qbcs[:, :, None].to_broadcast((128, H, BL)),
                                op=ALU.add)
        aps_cm.__exit__(None, None, None)

        # ---- einsum: 512 weight-stationary matmuls
        aps_cm = tc.tile_pool(name=f"attps{li}e", bufs=2, space="PSUM")
        aps = aps_cm.__enter__()
        apt_cm = tc.tile_pool(name=f"attps{li}t", bufs=2, space="PSUM")
        apt = apt_cm.__enter__()
        OutSel = apool.tile([128, NPAIR * BL], BF)        # [(r,o), (pair,b)]
        OutT = apool.tile([128, BL * H * E], BF)          # [(r,m), (b,h,o)]
        OT4 = OutT[:].rearrange("p (b h o) -> p b h o", b=BL, h=H)
        OS4 = OutSel[:].rearrange("p (pr b) -> p pr b", b=BL)
        for g4 in range(4):                                # psum bank groups
            ps_e = aps.tile([128, 512], F32, tag="eps")
            for bl in range(g4 * 8, (g4 + 1) * 8):         # 8 weight blocks of 16
                wsb = apool2.tile([128, 16 * 128], BF, tag="wsb")
                nc.sync.dma_start(wsb[:], W2_d[li, bl])
                for j in range(16):
                    p = bl * 16 + j
                    nc.tensor.matmul(ps_e[:, (p % 128) * 4:(p % 128) * 4 + 4],
                                     wsb[:, j * 128:(j + 1) * 128],
                                     Xstk[:, p * 4:(p + 1) * 4],
                                     start=True, stop=True)
            nc.vector.tensor_copy(OutSel[:, g4 * 512:(g4 + 1) * 512], ps_e[:])
            # transpose the two heads this group just produced into
            # OutT2[(r,m), (b, h, o)] (r via partition-offset evictions)
            for hh in (2 * g4, 2 * g4 + 1):
                for b in range(BL):
                    for r in range(2):
                        ps_t = apt.tile([64, 64], BF, tag="tps")
                        nc.tensor.transpose(
                            ps_t[:], OS4[r * 64:(r + 1) * 64,
                                         hh * 64:(hh + 1) * 64, b],
                            identb[r * 64:(r + 1) * 64, r * 64:(r + 1) * 64])
                        nc.vector.tensor_copy(OT4[r * 64:(r + 1) * 64, b, hh, :],
                                              ps_t[:])
        apt_cm.__exit__(None, None, None)
        aps_cm.__exit__(None, None, None)

        # ---- per-batch pipelined back-half:
        #      fused o-proj(b) -> decomp1(b) -> FFN(2b, 2b+1) -> decomp2(b)
        aps_cm = tc.tile_pool(name=f"attps{li}v", bufs=2, space="PSUM")
        aps = aps_cm.__enter__()
        fpool_cm = tc.tile_pool(name=f"ffn{li}", bufs=2)
        fpool = fpool_cm.__enter__()
        dpool_cm = tc.tile_pool(name=f"dc{li}", bufs=3)
        dpool = dpool_cm.__enter__()
        fps_cm = tc.tile_pool(name=f"ffnps{li}", bufs=3, space="PSUM")
        fps = fps_cm.__enter__()
        fpsg_cm = tc.tile_pool(name=f"ffnpsg{li}", bufs=2, space="PSUM")
        fpsg = fpsg_cm.__enter__()
        for b in range(BL):
            for j in range(2):
                for n in range(DCH):
                    ps_o = aps.tile([128, 512], F32, tag="ops")
                    nc.tensor.matmul(ps_o[:], m2s[:, j, n * 128:(n + 1) * 128],
                                     OutT[:, b * 512:(b + 1) * 512],
                                     start=True, stop=True)
                    xv = xT[:, n, b * L:(b + 1) * L].rearrange(
                        "p (h o two) -> p h o two", h=H, o=E, two=2)[:, :, :, j]
                    nc.vector.scalar_tensor_tensor(
                        out=xv, in0=ps_o[:].rearrange("p (h o) -> p h o", h=H),
                        scalar=1.0, in1=xv, op0=ALU.mult, op1=ALU.add,
                        accum_out=sl_z[:, n, b, li * 2 + j:li * 2 + j + 1])
            _decomp_b(nc, dpool, xT, sl_x, b, slot=li * 2 + 0)
            for s2 in (2 * b, 2 * b + 1):
                y1s = []
                for f in range(4):
                    y1 = fpool.tile([128, 4, 512], BF, tag=f"y1f{f}", name=f"y1f{f}")
                    y1s.append(y1)
                    for mm in range(4):
                        ps_f = fps.tile([128, 512], F32, tag="fps")
                        for k in range(DCH):
                            nc.tensor.matmul(
                                ps_f[:], c1s[:, k, f * 512 + mm * 128:f * 512 + (mm + 1) * 128],
                                xT[:, k, s2 * 512:(s2 + 1) * 512],
                                start=(k == 0), stop=(k == DCH - 1))
                        nc.scalar.activation(y1[:, mm, :], ps_f[:], ACTF.Gelu, bias=0.0)
                half = s2 % 2
                zslot = 4 + li * 2 + half
                for n in range(DCH):
                    ps_g = fpsg.tile([128, 512], F32, tag="gps")
                    for f in range(4):
                        for mm in range(4):
                            nc.tensor.matmul(ps_g[:],
                                             c2s[:, f * 4 + mm, n * 128:(n + 1) * 128],
                                             y1s[f][:, mm, :],
                                             start=(f == 0 and mm == 0),
                                             stop=(f == 3 and mm == 3))
                    xsl = xT[:, n, s2 * 512:(s2 + 1) * 512]
                    nc.vector.scalar_tensor_tensor(
                        out=xsl, in0=ps_g[:], scalar=1.0, in1=xsl,
                        op0=ALU.mult, op1=ALU.add,
                        accum_out=sl_z[:, n, b, zslot:zslot + 1])
            # decomp2 (skipped for the last layer: the final seasonal x4 only
            # enters the output via mean(x4) + mean(MA(z4)) = mean(z4))
            if li < NLAYERS - 1:
                _decomp_b(nc, dpool, xT, sl_x, b, slot=li * 2 + 1)
        fpsg_cm.__exit__(None, None, None)
        fps_cm.__exit__(None, None, None)
        dpool_cm.__exit__(None, None, None)
        fpool_cm.__exit__(None, None, None)
        aps_cm.__exit__(None, None, None)


def _decomp_b(nc, dpool, xT, sl_x, b, slot):
        if True:
            for cch in range(DCH):
                stripe = dpool.tile([128, SW], BF, tag="stripe")
                nc.scalar.copy(stripe[:, PAD + 1:PAD + 1 + L],
                               xT[:, cch, b * L:(b + 1) * L])
                nc.scalar.copy(
                    stripe[:, 0:PAD + 1],
                    xT[:, cch, b * L:b * L + 1].to_broadcast((128, PAD + 1)))
                nc.scalar.copy(
                    stripe[:, PAD + 1 + L:SW],
                    xT[:, cch, (b + 1) * L - 1:(b + 1) * L].to_broadcast((128, PAD)))
                init = dpool.tile([128, 1], F32, tag="init")
                nc.vector.reduce_sum(init[:], stripe[:, 0:K_MA], axis=AX.X)
                scano = dpool.tile([128, L], F32, tag="scano")
                nc.vector.tensor_tensor_scan(
                    scano[:], stripe[:, K_MA:SW], stripe[:, 0:L], init[:],
                    op0=ALU.add, op1=ALU.subtract)
                nc.vector.scalar_tensor_tensor(
                    out=xT[:, cch, b * L:(b + 1) * L], in0=scano[:],
                    scalar=-1.0 / K_MA, in1=xT[:, cch, b * L:(b + 1) * L],
                    op0=ALU.mult, op1=ALU.add,
                    accum_out=sl_x[:, cch, b, slot:slot + 1])


# ---------------------------------------------------------------------------
# entry point
# ---------------------------------------------------------------------------

_CACHE = {}


def kernel(x_enc, params, freq_index):
    consts = build_consts(params, freq_index)
    core_maps = stage_core_inputs(x_enc)
    if 'nc' not in _CACHE:
        _CACHE['nc'] = build_program()
    nc = _CACHE['nc']
    in_maps = [{**consts, **cm} for cm in core_maps]
    from concourse.bass_utils import run_bass_kernel_spmd
    res = run_bass_kernel_spmd(nc, in_maps, core_ids=list(range(NCORES)))
    out = np.concatenate([res.results[i]['out'] for i in range(NCORES)], axis=0)
    return out.astype(np.float32)
